# revision 27
# baseline (speedup 1.0000x reference)
"""PointNet++ backbone (nn_BackbonePointNet2) on 8 Trainium2 NeuronCores.

Sharding: data-parallel over batch. B=32 clouds -> 4 clouds per core.
Host precomputes index structure (FPS order, ball-query neighbor lists,
3-NN interpolation indices/weights) in numpy; the device kernel does all
dense compute: grouping gathers (gpsimd ap_gather), every shared-MLP layer
(TensorE matmuls, fp32 PSUM accum), fused BN+ReLU (ACT/DVE), max-pools
(DVE reduce), FP interpolation matmuls, heads with fused leaky-relu.

Activations on SBUF are logically [C, n] with C on partitions; C > 128 is
stored as [128, q, n] (q = C // 128).
"""

import numpy as np
import ml_dtypes

B, N = 32, 8192
T, CH = 5, 27
NCORES = 8
BPC = B // NCORES  # clouds per core

S1, K1, R1 = 256, 32, 0.05
S2, K2, R2 = 128, 64, 0.1
S3, K3, R3 = 32, 128, 0.2
G2 = S2 * K2   # 8192
G3 = S3 * K3   # 4096

BF16 = ml_dtypes.bfloat16

# K-block split per layer: (rows, bf16?) — order must match kernel rhs order.
_SPLITS = {
    "sa1": [[(6, 1)], [(32, 1)], [(32, 1)]],
    "sa2": [[(67, 0)], [(64, 1)], [(64, 1)]],
    "sa3": [[(128, 0), (3, 1)], [(128, 1)], [(128, 1)]],
    "sa4": [[(3, 1), (128, 1), (128, 1)], [(128, 1), (128, 1)],
            [(128, 1), (128, 1)]],
    "fp4": [[(128, 1)] * 6, [(128, 1), (128, 1)]],
    "fp3": [[(128, 1), (128, 1), (128, 0)], [(128, 1), (128, 1)]],
    "fp2": [[(128, 1), (128, 1), (64, 0)], [(128, 1), (128, 1)]],
    "fp1": [[(128, 1), (6, 1)], [(128, 1)]],
    "head1": [[(128, 1)]],
    "head2": [[(128, 1)]],
    "head3": [[(128, 1)]],
}
_COUT = {"sa1": [32, 32, 64], "sa2": [64, 64, 128], "sa3": [128, 128, 256],
         "sa4": [256, 256, 512], "fp4": [256, 256], "fp3": [256, 256],
         "fp2": [256, 128], "fp1": [128, 128], "head1": [128],
         "head2": [128], "head3": [CH * T]}

# ----------------------------------------------------------------- host math


def _fps(xyz, npoint):
    Bb, Nn, _ = xyz.shape
    dist = np.full((Bb, Nn), 1e10, np.float32)
    idxs = np.zeros((Bb, npoint), np.int64)
    last = np.zeros((Bb,), np.int64)
    ar = np.arange(Bb)
    for s in range(1, npoint):
        p = xyz[ar, last]
        d = ((xyz - p[:, None, :]) ** 2).sum(-1)
        dist = np.minimum(dist, d)
        last = dist.argmax(-1)
        idxs[:, s] = last
    return idxs


def _ball(new_xyz, xyz, r, k):
    d2 = ((new_xyz[:, None, :] - xyz[None, :, :]) ** 2).sum(-1)
    Nn = xyz.shape[0]
    keyv = np.where(d2 < r * r, np.arange(Nn)[None, :], Nn)
    part = np.partition(keyv, k - 1, axis=-1)[:, :k] if k < Nn else keyv
    order = np.sort(part, -1)[:, :k]
    first = order[:, :1]
    idx = np.where(order < Nn, order, np.where(first < Nn, first, 0))
    return idx.astype(np.int64)


def _interp_mat(unknown, known, n, m):
    d2 = ((unknown[:, None, :] - known[None, :, :]) ** 2).sum(-1)
    ii = np.argsort(d2, axis=-1, kind="stable")[:, :3]
    dsel = np.take_along_axis(d2, ii, -1).astype(np.float32)
    dist = np.sqrt(np.maximum(dsel, 0.0))
    w = 1.0 / (dist + 1e-8)
    w = (w / w.sum(-1, keepdims=True)).astype(np.float32)
    W = np.zeros((m, n), np.float32)
    ar = np.arange(n)
    for k in range(3):
        W[ii[:, k], ar] += w[:, k]
    return W


def _wrap16(idx, parts):
    n = idx.shape[0]
    w = idx.reshape(n // 16, 16).T.astype(np.int16)
    return np.tile(w, (parts // 16, 1))


def _prep_host(xyz, points, params):
    fidx1 = _fps(xyz, S1)
    l1x = np.take_along_axis(xyz, fidx1[..., None], axis=1)
    fidx2 = _fps(l1x, S2)
    l2x = np.take_along_axis(l1x, fidx2[..., None], axis=1)
    fidx3 = _fps(l2x, S3)
    l3x = np.take_along_axis(l2x, fidx3[..., None], axis=1)

    ptsT = np.transpose(points, (0, 2, 1))

    g1 = np.empty((B, 6, S1 * K1), np.float32)
    g2c = np.empty((B, 3, G2), np.float32)
    g3c = np.empty((B, 3, G3), np.float32)
    idx2w = np.empty((B, 4, 64, G2 // 64), np.int16)   # four quarters
    idx3w = np.empty((B, 128, G3 // 16), np.int16)
    wi3 = np.empty((B, S3, S2), np.float32)
    wi2 = np.empty((B, S2, S1), np.float32)
    wi1 = np.empty((B, S1, N), np.float32)
    for b in range(B):
        i1 = _ball(l1x[b], xyz[b], R1, K1)
        gx = xyz[b][i1] - l1x[b][:, None, :]
        gn = ptsT[b][i1]
        g1[b] = np.concatenate([gx, gn], -1).transpose(2, 0, 1).reshape(6, -1)

        i2 = _ball(l2x[b], l1x[b], R2, K2)
        i2f = i2.reshape(-1)
        for qq in range(4):
            idx2w[b, qq] = _wrap16(i2f[qq * G2 // 4:(qq + 1) * G2 // 4], 64)
        g2c[b] = (l1x[b][i2] - l2x[b][:, None, :]).transpose(2, 0, 1).reshape(3, -1)

        i3 = _ball(l3x[b], l2x[b], R3, K3)
        g3c[b] = (l2x[b][i3] - l3x[b][:, None, :]).transpose(2, 0, 1).reshape(3, -1)
        idx3w[b] = _wrap16(i3.reshape(-1), 128)

        wi3[b] = _interp_mat(l2x[b], l3x[b], S2, S3)
        wi2[b] = _interp_mat(l1x[b], l2x[b], S1, S2)
        wi1[b] = _interp_mat(xyz[b], l1x[b], N, S1)

    uf = np.concatenate([np.transpose(xyz, (0, 2, 1)), points], 1)

    wblobs = {}
    for nm, layers in _SPLITS.items():
        for li, blocks in enumerate(layers):
            W, g, bb = params[nm][li]
            W = np.asarray(W, np.float32) * np.asarray(g, np.float32)[:, None]
            WT = W.T.copy()
            if nm in ("sa2", "sa3") and li == 0:
                WT = np.concatenate([WT[3:], WT[:3]], 0)  # [feat; coord]
            r0 = 0
            for ki, (kb, isbf) in enumerate(blocks):
                blk = WT[r0:r0 + kb]
                wblobs[f"W_{nm}_{li}_{ki}"] = (
                    blk.astype(BF16) if isbf else blk.astype(np.float32))
                r0 += kb
            assert r0 == WT.shape[0], (nm, li, r0, WT.shape)
            bb = np.asarray(bb, np.float32)
            Cout = bb.shape[0]
            if Cout % 128 == 0 and Cout > 128:
                bmat = bb.reshape(Cout // 128, 128).T.copy()
            elif Cout < 128:
                bmat = np.tile(bb, 128 // Cout)[:128, None].copy()
            else:
                bmat = bb[:, None].copy()
            wblobs[f"b_{nm}_{li}"] = bmat

    per_core = []
    for c in range(NCORES):
        sl = slice(c * BPC, (c + 1) * BPC)
        m = {
            "g1": g1[sl].astype(BF16),
            "g2c": g2c[sl].astype(np.float32),
            "g3c": g3c[sl].astype(BF16),
            "idx2": idx2w[sl],
            "idx3": idx3w[sl],
            "l3x": l3x[sl].transpose(0, 2, 1).astype(BF16).copy(),
            "wi3": wi3[sl].astype(BF16),
            "wi2": wi2[sl].astype(BF16),
            "wi1": wi1[sl].reshape(BPC, 2, 128, N).astype(BF16),
            "uf": uf[sl].astype(BF16),
        }
        m.update(wblobs)
        per_core.append(m)
    return per_core


# ------------------------------------------------------------- device kernel

_BUILT = None
CNK = 512
CNB = 1024


def _build():
    import contextlib
    import concourse.mybir as mybir
    import concourse.tile as tile
    from concourse import bacc
    from concourse.masks import make_identity

    fp32 = mybir.dt.float32
    bf16 = mybir.dt.bfloat16
    i16 = mybir.dt.int16
    AX = mybir.AxisListType.X
    ALU = mybir.AluOpType
    ACTF = mybir.ActivationFunctionType

    nc = bacc.Bacc("TRN2", target_bir_lowering=False, debug=False)

    def din(name, shape, dt):
        return nc.dram_tensor(name, shape, dt, kind="ExternalInput").ap()

    g1_d = din("g1", [BPC, 6, S1 * K1], bf16)
    g2c_d = din("g2c", [BPC, 3, G2], fp32)
    g3c_d = din("g3c", [BPC, 3, G3], bf16)
    idx2_d = din("idx2", [BPC, 4, 64, G2 // 64], i16)
    idx3_d = din("idx3", [BPC, 128, G3 // 16], i16)
    l3x_d = din("l3x", [BPC, 3, S3], bf16)
    wi3_d = din("wi3", [BPC, S3, S2], bf16)
    wi2_d = din("wi2", [BPC, S2, S1], bf16)
    wi1_d = din("wi1", [BPC, 2, 128, N], bf16)
    uf_d = din("uf", [BPC, 6, N], bf16)
    out_d = nc.dram_tensor("out", [BPC, CH * T, N], fp32,
                           kind="ExternalOutput").ap()
    import os
    dbg = os.environ.get("BASSDBG", "0") == "1"
    dbg_d = {}
    if dbg:
        for nm_, shp, dt_ in [("d_l1f", [64, S1], fp32),
                              ("d_l2f", [128, S2], fp32),
                              ("d_l3f", [128, 2, S3], bf16),
                              ("d_l4f", [128, 4, 1], fp32),
                              ("d_l3fn", [128, 2, S3], bf16),
                              ("d_l2fn", [128, 2, S2], bf16),
                              ("d_l1fn", [128, S1], bf16),
                              ("d_h1fp1", [128, N], bf16),
                              ("d_feat", [128, N], bf16),
                              ("d_g2f", [64, G2 // 2], fp32),
                              ("d_t1", [128, N], bf16)]:
            dbg_d[nm_] = nc.dram_tensor(nm_, shp, dt_,
                                        kind="ExternalOutput").ap()

    wdecl = {}
    for nm, layers in _SPLITS.items():
        wdecl[nm] = []
        for li, blocks in enumerate(layers):
            Cout = _COUT[nm][li]
            blks = [din(f"W_{nm}_{li}_{ki}", [kb, Cout],
                        bf16 if isbf else fp32)
                    for ki, (kb, isbf) in enumerate(blocks)]
            bshape = ([128, Cout // 128] if (Cout % 128 == 0 and Cout > 128)
                      else [min(Cout, 128), 1])
            wdecl[nm].append((blks, din(f"b_{nm}_{li}", bshape, fp32), Cout))

    with tile.TileContext(nc) as tc:
        with contextlib.ExitStack() as ctx:
            wpool = ctx.enter_context(tc.tile_pool(name="w", bufs=1))
            cpool = ctx.enter_context(tc.tile_pool(name="const", bufs=1))
            gin = ctx.enter_context(tc.tile_pool(name="gin", bufs=1))
            act16 = ctx.enter_context(tc.tile_pool(name="act16", bufs=2))
            g2fp = ctx.enter_context(tc.tile_pool(name="g2f", bufs=1))
            small = ctx.enter_context(tc.tile_pool(name="small", bufs=2))
            wip = ctx.enter_context(tc.tile_pool(name="wi", bufs=2))
            wi1p = ctx.enter_context(tc.tile_pool(name="wi1", bufs=1))
            outp = ctx.enter_context(tc.tile_pool(name="outp", bufs=1))
            pbig = ctx.enter_context(tc.tile_pool(name="pbig", bufs=3,
                                                  space="PSUM"))
            psml = ctx.enter_context(tc.tile_pool(name="psml", bufs=2,
                                                  space="PSUM"))

            identb = cpool.tile([128, 128], bf16, tag="idbf")
            make_identity(nc, identb[:])
            zbf = cpool.tile([128, CNB], bf16, tag="zbf")
            nc.vector.memset(zbf[:], 0.0)

            wt = {}
            for nm, layers in wdecl.items():
                wt[nm] = []
                for li, (blks, bd, Cout) in enumerate(layers):
                    tl = []
                    for ki, wd_ in enumerate(blks):
                        t = wpool.tile(wd_.shape, wd_.dtype,
                                       tag=f"W{nm}{li}{ki}")
                        nc.sync.dma_start(t[:], wd_)
                        tl.append(t)
                    bt = wpool.tile(bd.shape, fp32, tag=f"b{nm}{li}")
                    nc.sync.dma_start(bt[:], bd)
                    wt[nm].append((tl, bt, Cout))

            # two rotating activation buffers (16KB slots)
            _rot = [0]

            def abuf(shape, dt):
                _rot[0] ^= 1
                tg = "actA" if _rot[0] else "actB"
                return act16.tile(shape, dt, tag=tg, name=tg)


            def dump(nm_, ap, c_):
                if dbg and c_ == 0:
                    nc.sync.dma_start(dbg_d[nm_], ap)
            def act_store(ps_ap, out_ap, bias_ap, alt, func, alpha):
                if func == ACTF.Lrelu:
                    if alt % 2 == 0:
                        nc.scalar.activation(out_ap, ps_ap, ACTF.Prelu,
                                             bias=bias_ap, alpha=float(alpha))
                    else:
                        scr = small.tile([128, CNB], bf16, tag="lrl",
                                         name="lrl")
                        sap = scr[:ps_ap.shape[0], :ps_ap.shape[-1]]
                        nc.vector.tensor_scalar_add(sap, ps_ap, bias_ap)
                        nc.vector.scalar_tensor_tensor(
                            out_ap, sap, float(alpha), sap,
                            op0=ALU.mult, op1=ALU.max)
                elif func == ACTF.Relu and alt % 2 == 1 and out_ap.dtype == bf16:
                    nc.vector.scalar_tensor_tensor(
                        out_ap, ps_ap, bias_ap,
                        zbf[:ps_ap.shape[0], :ps_ap.shape[-1]],
                        op0=ALU.add, op1=ALU.max)
                elif func is None:
                    nc.scalar.activation(out_ap, ps_ap, ACTF.Copy)
                else:
                    nc.scalar.activation(out_ap, ps_ap, func,
                                         bias=bias_ap, alpha=alpha)

            def layer(nm, li, rhs_fns, n, out_fn, func="relu"):
                blks, bt, Cout = wt[nm][li]
                fn = {"relu": ACTF.Relu, "lrelu": ACTF.Lrelu,
                      "none": None}[func]
                alpha = 0.2 if func == "lrelu" else 0.0
                alt = 0
                for mq in range((Cout + 127) // 128):
                    mw = min(128, Cout - mq * 128)
                    for cb in range((n + CNB - 1) // CNB):
                        c0 = cb * CNB
                        cw = min(CNB, n - c0)
                        ps = pbig.tile([128, CNB], fp32, tag="ps")
                        for sub in range(0, cw, CNK):
                            sw = min(CNK, cw - sub)
                            sl = slice(c0 + sub, c0 + sub + sw)
                            for ki, rf in enumerate(rhs_fns):
                                nc.tensor.matmul(
                                    ps[:mw, sub:sub + sw],
                                    blks[ki][:, mq * 128:mq * 128 + mw],
                                    rf(sl), start=(ki == 0),
                                    stop=(ki == len(rhs_fns) - 1))
                        bap = (bt[:mw, mq:mq + 1] if bt.shape[1] > 1
                               else bt[:mw, :])
                        act_store(ps[:mw, :cw], out_fn(mq, slice(c0, c0 + cw)),
                                  bap, alt, fn, alpha)
                        alt += 1

            for c in range(BPC):
                # ======== SA1 ========
                g1t = gin.tile([6, S1 * K1], bf16, tag="gbuf")
                nc.sync.dma_start(g1t[:], g1_d[c])
                h1 = abuf([32, N], bf16)
                layer("sa1", 0, [lambda sl: g1t[:, sl]], N,
                      lambda mq, sl: h1[:, sl])
                h2 = abuf([32, N], bf16)
                layer("sa1", 1, [lambda sl: h1[:, sl]], N,
                      lambda mq, sl: h2[:, sl])
                h3 = abuf([64, N], bf16)
                layer("sa1", 2, [lambda sl: h2[:, sl]], N,
                      lambda mq, sl: h3[:, sl])
                l1fb = small.tile([64, S1], bf16, tag="l1fb")
                nc.vector.tensor_reduce(
                    l1fb[:], h3[:].rearrange("p (s k) -> p s k", k=K1),
                    axis=AX, op=ALU.max)
                l1f = small.tile([64, S1], fp32, tag="l1f")
                nc.vector.tensor_copy(l1f[:], l1fb[:])
                dump("d_l1f", l1f[:], c)

                # ======== SA2 ========
                i2t = small.tile([64, 4, G2 // 64], i16, tag="i2")
                nc.sync.dma_start(i2t[:],
                                  idx2_d[c].rearrange("h p n -> p h n"))
                h1 = abuf([64, G2], bf16)
                blks2, bt2, _ = wt["sa2"][0]
                for hf in range(4):
                    off = hf * (G2 // 4)
                    g2f = g2fp.tile([67, G2 // 4], fp32, tag="g2f")
                    nc.gpsimd.ap_gather(
                        g2f[0:64, :].rearrange("p (n d) -> p n d", d=1),
                        l1f[:].rearrange("p (n d) -> p n d", d=1),
                        i2t[:, hf, :], channels=64, num_elems=S1, d=1,
                        num_idxs=G2 // 4)
                    nc.sync.dma_start(g2f[64:67, :],
                                      g2c_d[c, :, off:off + G2 // 4])
                    if hf == 0:
                        dump("d_g2f", g2f[0:64, :], c)
                    for cb in range(2):
                        ps = pbig.tile([128, CNB], fp32, tag="ps")
                        for sub in range(2):
                            s0 = cb * CNB + sub * CNK
                            nc.tensor.matmul(
                                ps[:64, sub * CNK:(sub + 1) * CNK],
                                blks2[0][:], g2f[:, s0:s0 + CNK],
                                start=True, stop=True)
                        act_store(ps[:64, :], h1[:, off + cb * CNB:
                                                 off + (cb + 1) * CNB],
                                  bt2[:64, :], cb, ACTF.Relu, 0.0)
                h2 = abuf([64, G2], bf16)
                layer("sa2", 1, [lambda sl: h1[:, sl]], G2,
                      lambda mq, sl: h2[:, sl])
                h3 = abuf([128, G2], bf16)
                layer("sa2", 2, [lambda sl: h2[:, sl]], G2,
                      lambda mq, sl: h3[:, sl])
                l2fb = small.tile([128, S2], bf16, tag="l2fb")
                nc.vector.tensor_reduce(
                    l2fb[:], h3[:].rearrange("p (s k) -> p s k", k=K2),
                    axis=AX, op=ALU.max)
                l2f = small.tile([128, S2], fp32, tag="l2f")
                nc.vector.tensor_copy(l2f[:], l2fb[:])
                dump("d_l2f", l2f[:], c)

                # ======== SA3 ========
                i3t = small.tile([128, G3 // 16], i16, tag="i3")
                nc.sync.dma_start(i3t[:], idx3_d[c])
                g3f = g2fp.tile([128, G3], fp32, tag="g3f")
                nc.gpsimd.ap_gather(
                    g3f[:].rearrange("p (n d) -> p n d", d=1),
                    l2f[:].rearrange("p (n d) -> p n d", d=1),
                    i3t[:], channels=128, num_elems=S2, d=1, num_idxs=G3)
                g3ct = small.tile([3, G3], bf16, tag="g3c")
                nc.sync.dma_start(g3ct[:], g3c_d[c])
                h1 = abuf([128, G3], bf16)
                layer("sa3", 0, [lambda sl: g3f[:, sl],
                                 lambda sl: g3ct[:, sl]], G3,
                      lambda mq, sl: h1[:, sl])
                h2 = abuf([128, G3], bf16)
                layer("sa3", 1, [lambda sl: h1[:, sl]], G3,
                      lambda mq, sl: h2[:, sl])
                h3 = abuf([128, 2, G3], bf16)
                layer("sa3", 2, [lambda sl: h2[:, sl]], G3,
                      lambda mq, sl: h3[:, mq, sl])
                l3f = small.tile([128, 2, S3], bf16, tag="l3f")
                for q in range(2):
                    nc.vector.tensor_reduce(
                        l3f[:, q, :],
                        h3[:, q, :].rearrange("p (s k) -> p s k", k=K3),
                        axis=AX, op=ALU.max)
                dump("d_l3f", l3f[:], c)

                # ======== SA4 ========
                l3xt = small.tile([3, S3], bf16, tag="l3x")
                nc.sync.dma_start(l3xt[:], l3x_d[c])
                h1 = small.tile([128, 2, S3], bf16, tag="s4h1")
                layer("sa4", 0, [lambda sl: l3xt[:, sl],
                                 lambda sl: l3f[:, 0, sl],
                                 lambda sl: l3f[:, 1, sl]], S3,
                      lambda mq, sl: h1[:, mq, sl])
                h2 = small.tile([128, 2, S3], bf16, tag="s4h2")
                layer("sa4", 1, [lambda sl: h1[:, 0, sl],
                                 lambda sl: h1[:, 1, sl]], S3,
                      lambda mq, sl: h2[:, mq, sl])
                h4 = small.tile([128, 4, S3], bf16, tag="s4h3")
                layer("sa4", 2, [lambda sl: h2[:, 0, sl],
                                 lambda sl: h2[:, 1, sl]], S3,
                      lambda mq, sl: h4[:, mq, sl])
                l4f = small.tile([128, 4, 1], fp32, tag="l4f")
                for q in range(4):
                    nc.vector.tensor_reduce(
                        l4f[:, q, :], h4[:, q, :], axis=AX, op=ALU.max)
                dump("d_l4f", l4f[:], c)

                # ======== FP4 ========
                b4 = small.tile([128, 4, S3], bf16, tag="b4")
                for q in range(4):
                    nc.vector.tensor_scalar_add(
                        b4[:, q, :], zbf[:, :S3], l4f[:, q, :])
                h1 = small.tile([128, 2, S3], bf16, tag="f4h1")
                layer("fp4", 0,
                      [lambda sl, q=q: b4[:, q, sl] for q in range(4)]
                      + [lambda sl: l3f[:, 0, sl], lambda sl: l3f[:, 1, sl]],
                      S3, lambda mq, sl: h1[:, mq, sl])
                l3fn = small.tile([128, 2, S3], bf16, tag="l3fn")
                layer("fp4", 1, [lambda sl: h1[:, 0, sl],
                                 lambda sl: h1[:, 1, sl]], S3,
                      lambda mq, sl: l3fn[:, mq, sl])
                dump("d_l3fn", l3fn[:], c)

                # ======== FP3 ========
                wi3t = wip.tile([S3, S2], bf16, tag="wi3")
                nc.sync.dma_start(wi3t[:], wi3_d[c])
                l3fT = small.tile([S3, 256], bf16, tag="l3fT")
                for q in range(2):
                    pt = psml.tile([128, CNK], bf16, tag="psm")
                    nc.tensor.transpose(pt[:S3, :128], l3fn[:, q, :],
                                        identb[:])
                    nc.scalar.activation(l3fT[:, q * 128:(q + 1) * 128],
                                         pt[:S3, :128], ACTF.Copy)
                it3 = small.tile([128, 2, S2], bf16, tag="it3")
                for q in range(2):
                    ps = psml.tile([128, CNK], fp32, tag="psm")
                    nc.tensor.matmul(ps[:, :S2],
                                     l3fT[:, q * 128:(q + 1) * 128],
                                     wi3t[:], start=True, stop=True)
                    nc.scalar.activation(it3[:, q, :], ps[:, :S2], ACTF.Copy)
                h1 = small.tile([128, 2, S2], bf16, tag="f3h1")
                layer("fp3", 0, [lambda sl: it3[:, 0, sl],
                                 lambda sl: it3[:, 1, sl],
                                 lambda sl: l2f[:, sl]], S2,
                      lambda mq, sl: h1[:, mq, sl])
                l2fn = small.tile([128, 2, S2], bf16, tag="l2fn")
                layer("fp3", 1, [lambda sl: h1[:, 0, sl],
                                 lambda sl: h1[:, 1, sl]], S2,
                      lambda mq, sl: l2fn[:, mq, sl])
                dump("d_l2fn", l2fn[:], c)

                # ======== FP2 ========
                wi2t = wip.tile([S2, S1], bf16, tag="wi2")
                nc.sync.dma_start(wi2t[:], wi2_d[c])
                l2fT = small.tile([S2, 256], bf16, tag="l2fT")
                for q in range(2):
                    pt = psml.tile([128, CNK], bf16, tag="psm")
                    nc.tensor.transpose(pt[:S2, :128], l2fn[:, q, :],
                                        identb[:])
                    nc.scalar.activation(l2fT[:, q * 128:(q + 1) * 128],
                                         pt[:S2, :128], ACTF.Copy)
                it2 = small.tile([128, 2, S1], bf16, tag="it2")
                for q in range(2):
                    ps = psml.tile([128, CNK], fp32, tag="psm")
                    nc.tensor.matmul(ps[:, :S1],
                                     l2fT[:, q * 128:(q + 1) * 128],
                                     wi2t[:], start=True, stop=True)
                    nc.scalar.activation(it2[:, q, :], ps[:, :S1], ACTF.Copy)
                h1 = small.tile([128, 2, S1], bf16, tag="f2h1")
                layer("fp2", 0, [lambda sl: it2[:, 0, sl],
                                 lambda sl: it2[:, 1, sl],
                                 lambda sl: l1f[:, sl]], S1,
                      lambda mq, sl: h1[:, mq, sl])
                l1fn = small.tile([128, S1], bf16, tag="l1fn")
                layer("fp2", 1, [lambda sl: h1[:, 0, sl],
                                 lambda sl: h1[:, 1, sl]], S1,
                      lambda mq, sl: l1fn[:, sl])
                dump("d_l1fn", l1fn[:], c)

                # ======== FP1 (interp matmul folded into layer 0) ========
                blks, b0t, _ = wt["fp1"][0]
                w0a, w0b = blks
                psy = psml.tile([128, CNK], fp32, tag="psm")
                nc.tensor.matmul(psy[:, :S1], w0a[:], l1fn[:],
                                 start=True, stop=True)
                ysb = small.tile([128, S1], bf16, tag="ysb")
                nc.scalar.activation(ysb[:], psy[:, :S1], ACTF.Copy)
                yT = small.tile([128, 2, 128], bf16, tag="yT")
                for q in range(2):
                    pt = psml.tile([128, CNK], bf16, tag="psm")
                    nc.tensor.transpose(pt[:, :128],
                                        ysb[:, q * 128:(q + 1) * 128],
                                        identb[:])
                    nc.scalar.activation(yT[:, q, :], pt[:, :128], ACTF.Copy)
                wi1t = wi1p.tile([128, 2, N // 2], bf16, tag="wi1")
                nc.sync.dma_start(wi1t[:], wi1_d[c, :, :, :N // 2])
                wi1u = wi1p.tile([128, 2, N // 2], bf16, tag="wi1b")
                nc.sync.dma_start(wi1u[:], wi1_d[c, :, :, N // 2:])
                uft = gin.tile([6, N], bf16, tag="gbuf")
                nc.sync.dma_start(uft[:], uf_d[c])
                h1 = abuf([128, N], bf16)
                for cb in range(N // CNK):
                    sl = slice(cb * CNK, (cb + 1) * CNK)
                    wsrc = wi1t if cb < 8 else wi1u
                    slw = slice((cb % 8) * CNK, (cb % 8 + 1) * CNK)
                    ps = pbig.tile([128, CNK], fp32, tag="ps")
                    nc.tensor.matmul(ps[:, :], yT[:, 0, :], wsrc[:, 0, slw],
                                     start=True, stop=False)
                    nc.tensor.matmul(ps[:, :], yT[:, 1, :], wsrc[:, 1, slw],
                                     start=False, stop=False)
                    nc.tensor.matmul(ps[:, :], w0b[:], uft[:, sl],
                                     start=False, stop=True)
                    act_store(ps[:, :], h1[:, sl], b0t[:, :], cb,
                              ACTF.Relu, 0.0)
                dump("d_h1fp1", h1[:], c)
                feat = abuf([128, N], bf16)
                layer("fp1", 1, [lambda sl: h1[:, sl]], N,
                      lambda mq, sl: feat[:, sl])
                dump("d_feat", feat[:], c)

                # ======== heads ========
                t1 = abuf([128, N], bf16)
                layer("head1", 0, [lambda sl: feat[:, sl]], N,
                      lambda mq, sl: t1[:, sl], func="lrelu")
                dump("d_t1", t1[:], c)
                t2 = abuf([128, N], bf16)
                layer("head2", 0, [lambda sl: t1[:, sl]], N,
                      lambda mq, sl: t2[:, sl], func="lrelu")
                w3 = wt["head3"][0][0][0]
                for cb in range(N // CNB):
                    bsl = slice(cb * CNB, (cb + 1) * CNB)
                    ps = pbig.tile([128, CNB], fp32, tag="ps")
                    for sub in range(2):
                        s0 = cb * CNB + sub * CNK
                        nc.tensor.matmul(ps[:, sub * CNK:(sub + 1) * CNK],
                                         w3[:, 0:128], t2[:, s0:s0 + CNK],
                                         start=True, stop=True)
                    osb = outp.tile([128, CNB], fp32, tag="osb")
                    if cb % 2 == 0:
                        nc.scalar.activation(osb[:], ps[:, :], ACTF.Copy)
                    else:
                        nc.vector.tensor_copy(osb[:], ps[:, :])
                    nc.sync.dma_start(out_d[c, 0:128, bsl], osb[:])
                    for sub in range(2):
                        s0 = cb * CNB + sub * CNK
                        ps2 = psml.tile([128, CNK], fp32, tag="psm",
                                        name="ps2")
                        nc.tensor.matmul(ps2[:7, :], w3[:, 128:135],
                                         t2[:, s0:s0 + CNK],
                                         start=True, stop=True)
                        osb2 = outp.tile([7, CNK], fp32, tag="osb2")
                        nc.vector.tensor_copy(osb2[:], ps2[:7, :])
                        nc.sync.dma_start(out_d[c, 128:135, s0:s0 + CNK],
                                          osb2[:])

    nc.compile()
    return nc


def _get_built():
    global _BUILT
    if _BUILT is None:
        _BUILT = _build()
    return _BUILT


def run_device(per_core, trace=False, tmpdir=None):
    from concourse.bass_utils import run_bass_kernel_spmd

    nc = _get_built()
    res = run_bass_kernel_spmd(nc, per_core, core_ids=list(range(NCORES)),
                               trace=trace, tmpdir=tmpdir)
    out = np.concatenate([r["out"] for r in res.results], 0)
    return out, res


def kernel(xyz, points, params):
    xyz = np.asarray(xyz, np.float32)
    points = np.asarray(points, np.float32)
    per_core = _prep_host(xyz, points, params)
    out, _ = run_device(per_core)
    bat = out.transpose(0, 2, 1).reshape(B, N, CH, T)
    return bat[:, :, :2, :], bat[:, :, 2:-20, :], bat[:, :, -20:, :]


# revision 34
# speedup vs baseline: 1.0115x; 1.0115x over previous
"""PointNet++ backbone (nn_BackbonePointNet2) on 8 Trainium2 NeuronCores.

Sharding: data-parallel over batch. B=32 clouds -> 4 clouds per core.
Host precomputes index structure (FPS order, ball-query neighbor lists,
3-NN interpolation indices/weights) in numpy; the device kernel does all
dense compute: grouping gathers (gpsimd ap_gather), every shared-MLP layer
(TensorE matmuls, fp32 PSUM accum), fused BN+ReLU (ACT/DVE), max-pools
(DVE reduce), FP interpolation matmuls, heads with fused leaky-relu.

Activations on SBUF are logically [C, n] with C on partitions; C > 128 is
stored as [128, q, n] (q = C // 128).
"""

import numpy as np
import ml_dtypes

B, N = 32, 8192
T, CH = 5, 27
NCORES = 8
BPC = B // NCORES  # clouds per core

S1, K1, R1 = 256, 32, 0.05
S2, K2, R2 = 128, 64, 0.1
S3, K3, R3 = 32, 128, 0.2
G2 = S2 * K2   # 8192
G3 = S3 * K3   # 4096

BF16 = ml_dtypes.bfloat16

# K-block split per layer: (rows, bf16?) — order must match kernel rhs order.
_SPLITS = {
    "sa1": [[(6, 1)], [(32, 1)], [(32, 1)]],
    "sa2": [[(67, 0)], [(64, 1)], [(64, 1)]],
    "sa3": [[(128, 0), (3, 1)], [(128, 1)], [(128, 1)]],
    "sa4": [[(3, 1), (128, 1), (128, 1)], [(128, 1), (128, 1)],
            [(128, 1), (128, 1)]],
    "fp4": [[(128, 1)] * 6, [(128, 1), (128, 1)]],
    "fp3": [[(128, 1), (128, 1), (128, 0)], [(128, 1), (128, 1)]],
    "fp2": [[(128, 1), (128, 1), (64, 0)], [(128, 1), (128, 1)]],
    "fp1": [[(128, 1), (6, 1)], [(128, 1)]],
    "head1": [[(128, 1)]],
    "head2": [[(128, 1)]],
    "head3": [[(128, 1)]],
}
_COUT = {"sa1": [32, 32, 64], "sa2": [64, 64, 128], "sa3": [128, 128, 256],
         "sa4": [256, 256, 512], "fp4": [256, 256], "fp3": [256, 256],
         "fp2": [256, 128], "fp1": [128, 128], "head1": [128],
         "head2": [128], "head3": [CH * T]}

# ----------------------------------------------------------------- host math


def _fps(xyz, npoint):
    Bb, Nn, _ = xyz.shape
    dist = np.full((Bb, Nn), 1e10, np.float32)
    idxs = np.zeros((Bb, npoint), np.int64)
    last = np.zeros((Bb,), np.int64)
    ar = np.arange(Bb)
    for s in range(1, npoint):
        p = xyz[ar, last]
        d = ((xyz - p[:, None, :]) ** 2).sum(-1)
        dist = np.minimum(dist, d)
        last = dist.argmax(-1)
        idxs[:, s] = last
    return idxs


def _ball(new_xyz, xyz, r, k):
    d2 = ((new_xyz[:, None, :] - xyz[None, :, :]) ** 2).sum(-1)
    Nn = xyz.shape[0]
    keyv = np.where(d2 < r * r, np.arange(Nn)[None, :], Nn)
    part = np.partition(keyv, k - 1, axis=-1)[:, :k] if k < Nn else keyv
    order = np.sort(part, -1)[:, :k]
    first = order[:, :1]
    idx = np.where(order < Nn, order, np.where(first < Nn, first, 0))
    return idx.astype(np.int64)


def _interp_mat(unknown, known, n, m):
    d2 = ((unknown[:, None, :] - known[None, :, :]) ** 2).sum(-1)
    ii = np.argsort(d2, axis=-1, kind="stable")[:, :3]
    dsel = np.take_along_axis(d2, ii, -1).astype(np.float32)
    dist = np.sqrt(np.maximum(dsel, 0.0))
    w = 1.0 / (dist + 1e-8)
    w = (w / w.sum(-1, keepdims=True)).astype(np.float32)
    W = np.zeros((m, n), np.float32)
    ar = np.arange(n)
    for k in range(3):
        W[ii[:, k], ar] += w[:, k]
    return W


def _wrap16(idx, parts):
    n = idx.shape[0]
    w = idx.reshape(n // 16, 16).T.astype(np.int16)
    return np.tile(w, (parts // 16, 1))


def _prep_host(xyz, points, params):
    fidx1 = _fps(xyz, S1)
    l1x = np.take_along_axis(xyz, fidx1[..., None], axis=1)
    fidx2 = _fps(l1x, S2)
    l2x = np.take_along_axis(l1x, fidx2[..., None], axis=1)
    fidx3 = _fps(l2x, S3)
    l3x = np.take_along_axis(l2x, fidx3[..., None], axis=1)

    ptsT = np.transpose(points, (0, 2, 1))

    g1 = np.empty((B, 6, S1 * K1), np.float32)
    g2c = np.empty((B, 3, G2), np.float32)
    g3c = np.empty((B, 3, G3), np.float32)
    idx2w = np.empty((B, 4, 64, G2 // 64), np.int16)   # four quarters
    idx3w = np.empty((B, 2, 128, G3 // 32), np.int16)
    wi3 = np.empty((B, S3, S2), np.float32)
    wi2 = np.empty((B, S2, S1), np.float32)
    wi1 = np.empty((B, S1, N), np.float32)
    for b in range(B):
        i1 = _ball(l1x[b], xyz[b], R1, K1)
        gx = xyz[b][i1] - l1x[b][:, None, :]
        gn = ptsT[b][i1]
        g1[b] = np.concatenate([gx, gn], -1).transpose(2, 0, 1).reshape(6, -1)

        i2 = _ball(l2x[b], l1x[b], R2, K2)
        i2f = i2.reshape(-1)
        for qq in range(4):
            idx2w[b, qq] = _wrap16(i2f[qq * G2 // 4:(qq + 1) * G2 // 4], 64)
        g2c[b] = (l1x[b][i2] - l2x[b][:, None, :]).transpose(2, 0, 1).reshape(3, -1)

        i3 = _ball(l3x[b], l2x[b], R3, K3)
        g3c[b] = (l2x[b][i3] - l3x[b][:, None, :]).transpose(2, 0, 1).reshape(3, -1)
        i3f = i3.reshape(-1)
        idx3w[b, 0] = _wrap16(i3f[:G3 // 2], 128)
        idx3w[b, 1] = _wrap16(i3f[G3 // 2:], 128)

        wi3[b] = _interp_mat(l2x[b], l3x[b], S2, S3)
        wi2[b] = _interp_mat(l1x[b], l2x[b], S1, S2)
        wi1[b] = _interp_mat(xyz[b], l1x[b], N, S1)

    uf = np.concatenate([np.transpose(xyz, (0, 2, 1)), points], 1)

    wblobs = {}
    for nm, layers in _SPLITS.items():
        for li, blocks in enumerate(layers):
            W, g, bb = params[nm][li]
            W = np.asarray(W, np.float32) * np.asarray(g, np.float32)[:, None]
            WT = W.T.copy()
            if nm in ("sa2", "sa3") and li == 0:
                WT = np.concatenate([WT[3:], WT[:3]], 0)  # [feat; coord]
            r0 = 0
            for ki, (kb, isbf) in enumerate(blocks):
                blk = WT[r0:r0 + kb]
                wblobs[f"W_{nm}_{li}_{ki}"] = (
                    blk.astype(BF16) if isbf else blk.astype(np.float32))
                r0 += kb
            assert r0 == WT.shape[0], (nm, li, r0, WT.shape)
            bb = np.asarray(bb, np.float32)
            Cout = bb.shape[0]
            if Cout % 128 == 0 and Cout > 128:
                bmat = bb.reshape(Cout // 128, 128).T.copy()
            elif Cout < 128:
                bmat = np.tile(bb, 128 // Cout)[:128, None].copy()
            else:
                bmat = bb[:, None].copy()
            wblobs[f"b_{nm}_{li}"] = bmat

    per_core = []
    for c in range(NCORES):
        sl = slice(c * BPC, (c + 1) * BPC)
        m = {
            "g1": g1[sl].astype(BF16),
            "g2c": g2c[sl].astype(np.float32),
            "g3c": g3c[sl].astype(BF16),
            "idx2": idx2w[sl],
            "idx3": idx3w[sl],
            "l3x": l3x[sl].transpose(0, 2, 1).astype(BF16).copy(),
            "wi3": wi3[sl].astype(BF16),
            "wi2": wi2[sl].astype(BF16),
            "wi1": wi1[sl].reshape(BPC, 2, 128, N).astype(BF16),
            "uf": uf[sl].astype(BF16),
        }
        m.update(wblobs)
        per_core.append(m)
    return per_core


# ------------------------------------------------------------- device kernel

_BUILT = None
CNK = 512
CNB = 1024


def _build():
    import contextlib
    import concourse.mybir as mybir
    import concourse.tile as tile
    from concourse import bacc
    from concourse.masks import make_identity

    fp32 = mybir.dt.float32
    bf16 = mybir.dt.bfloat16
    i16 = mybir.dt.int16
    AX = mybir.AxisListType.X
    ALU = mybir.AluOpType
    ACTF = mybir.ActivationFunctionType

    nc = bacc.Bacc("TRN2", target_bir_lowering=False, debug=False)

    def din(name, shape, dt):
        return nc.dram_tensor(name, shape, dt, kind="ExternalInput").ap()

    g1_d = din("g1", [BPC, 6, S1 * K1], bf16)
    g2c_d = din("g2c", [BPC, 3, G2], fp32)
    g3c_d = din("g3c", [BPC, 3, G3], bf16)
    idx2_d = din("idx2", [BPC, 4, 64, G2 // 64], i16)
    idx3_d = din("idx3", [BPC, 2, 128, G3 // 32], i16)
    l3x_d = din("l3x", [BPC, 3, S3], bf16)
    wi3_d = din("wi3", [BPC, S3, S2], bf16)
    wi2_d = din("wi2", [BPC, S2, S1], bf16)
    wi1_d = din("wi1", [BPC, 2, 128, N], bf16)
    uf_d = din("uf", [BPC, 6, N], bf16)
    out_d = nc.dram_tensor("out", [BPC, CH * T, N], fp32,
                           kind="ExternalOutput").ap()
    import os
    dbg = os.environ.get("BASSDBG", "0") == "1"
    dbg_d = {}
    if dbg:
        for nm_, shp, dt_ in [("d_l1f", [64, S1], fp32),
                              ("d_l2f", [128, S2], fp32),
                              ("d_l3f", [128, 2, S3], bf16),
                              ("d_l4f", [128, 4, 1], fp32),
                              ("d_l3fn", [128, 2, S3], bf16),
                              ("d_l2fn", [128, 2, S2], bf16),
                              ("d_l1fn", [128, S1], bf16),
                              ("d_h1fp1", [128, N], bf16),
                              ("d_feat", [128, N], bf16),
                              ("d_g2f", [64, G2 // 2], fp32),
                              ("d_t1", [128, N], bf16)]:
            dbg_d[nm_] = nc.dram_tensor(nm_, shp, dt_,
                                        kind="ExternalOutput").ap()

    wdecl = {}
    for nm, layers in _SPLITS.items():
        wdecl[nm] = []
        for li, blocks in enumerate(layers):
            Cout = _COUT[nm][li]
            blks = [din(f"W_{nm}_{li}_{ki}", [kb, Cout],
                        bf16 if isbf else fp32)
                    for ki, (kb, isbf) in enumerate(blocks)]
            bshape = ([128, Cout // 128] if (Cout % 128 == 0 and Cout > 128)
                      else [min(Cout, 128), 1])
            wdecl[nm].append((blks, din(f"b_{nm}_{li}", bshape, fp32), Cout))

    with tile.TileContext(nc) as tc:
        with contextlib.ExitStack() as ctx:
            wpool = ctx.enter_context(tc.tile_pool(name="w", bufs=1))
            cpool = ctx.enter_context(tc.tile_pool(name="const", bufs=1))
            gin = ctx.enter_context(tc.tile_pool(name="gin", bufs=1))
            act16 = ctx.enter_context(tc.tile_pool(name="act16", bufs=2))
            g2fp = ctx.enter_context(tc.tile_pool(name="g2f", bufs=1))
            small = ctx.enter_context(tc.tile_pool(name="small", bufs=2))
            wip = ctx.enter_context(tc.tile_pool(name="wi", bufs=2))
            wi1p = ctx.enter_context(tc.tile_pool(name="wi1", bufs=1))
            outp = ctx.enter_context(tc.tile_pool(name="outp", bufs=1))
            pbig = ctx.enter_context(tc.tile_pool(name="pbig", bufs=3,
                                                  space="PSUM"))
            psml = ctx.enter_context(tc.tile_pool(name="psml", bufs=2,
                                                  space="PSUM"))

            identb = cpool.tile([128, 128], bf16, tag="idbf")
            make_identity(nc, identb[:])
            zbf = cpool.tile([128, CNB], bf16, tag="zbf")
            nc.vector.memset(zbf[:], 0.0)

            wt = {}
            for nm, layers in wdecl.items():
                wt[nm] = []
                for li, (blks, bd, Cout) in enumerate(layers):
                    tl = []
                    for ki, wd_ in enumerate(blks):
                        t = wpool.tile(wd_.shape, wd_.dtype,
                                       tag=f"W{nm}{li}{ki}")
                        nc.sync.dma_start(t[:], wd_)
                        tl.append(t)
                    bt = wpool.tile(bd.shape, fp32, tag=f"b{nm}{li}")
                    nc.sync.dma_start(bt[:], bd)
                    wt[nm].append((tl, bt, Cout))

            # two rotating activation buffers (16KB slots)
            _rot = [0]

            def abuf(shape, dt):
                _rot[0] ^= 1
                tg = "actA" if _rot[0] else "actB"
                return act16.tile(shape, dt, tag=tg, name=tg)


            def dump(nm_, ap, c_):
                if dbg and c_ == 0:
                    nc.sync.dma_start(dbg_d[nm_], ap)
            def act_store(ps_ap, out_ap, bias_ap, alt, func, alpha):
                if func == ACTF.Lrelu:
                    if alt % 2 == 0:
                        nc.scalar.activation(out_ap, ps_ap, ACTF.Prelu,
                                             bias=bias_ap, alpha=float(alpha))
                    else:
                        scr = small.tile([128, CNB], bf16, tag="lrl",
                                         name="lrl")
                        sap = scr[:ps_ap.shape[0], :ps_ap.shape[-1]]
                        nc.vector.tensor_scalar_add(sap, ps_ap, bias_ap)
                        nc.vector.scalar_tensor_tensor(
                            out_ap, sap, float(alpha), sap,
                            op0=ALU.mult, op1=ALU.max)
                elif func == ACTF.Relu and alt % 2 == 1 and out_ap.dtype == bf16:
                    nc.vector.scalar_tensor_tensor(
                        out_ap, ps_ap, bias_ap,
                        zbf[:ps_ap.shape[0], :ps_ap.shape[-1]],
                        op0=ALU.add, op1=ALU.max)
                elif func is None:
                    nc.scalar.activation(out_ap, ps_ap, ACTF.Copy)
                else:
                    nc.scalar.activation(out_ap, ps_ap, func,
                                         bias=bias_ap, alpha=alpha)

            def layer(nm, li, rhs_fns, n, out_fn, func="relu"):
                blks, bt, Cout = wt[nm][li]
                fn = {"relu": ACTF.Relu, "lrelu": ACTF.Lrelu,
                      "none": None}[func]
                alpha = 0.2 if func == "lrelu" else 0.0
                alt = 0
                for mq in range((Cout + 127) // 128):
                    mw = min(128, Cout - mq * 128)
                    for cb in range((n + CNB - 1) // CNB):
                        c0 = cb * CNB
                        cw = min(CNB, n - c0)
                        ps = pbig.tile([128, CNB], fp32, tag="ps")
                        for sub in range(0, cw, CNK):
                            sw = min(CNK, cw - sub)
                            sl = slice(c0 + sub, c0 + sub + sw)
                            for ki, rf in enumerate(rhs_fns):
                                nc.tensor.matmul(
                                    ps[:mw, sub:sub + sw],
                                    blks[ki][:, mq * 128:mq * 128 + mw],
                                    rf(sl), start=(ki == 0),
                                    stop=(ki == len(rhs_fns) - 1))
                        bap = (bt[:mw, mq:mq + 1] if bt.shape[1] > 1
                               else bt[:mw, :])
                        act_store(ps[:mw, :cw], out_fn(mq, slice(c0, c0 + cw)),
                                  bap, alt, fn, alpha)
                        alt += 1

            for c in range(BPC):
                # ======== SA1 ========
                g1t = gin.tile([6, S1 * K1], bf16, tag="gbuf")
                nc.sync.dma_start(g1t[:], g1_d[c])
                h1 = abuf([32, N], bf16)
                layer("sa1", 0, [lambda sl: g1t[:, sl]], N,
                      lambda mq, sl: h1[:, sl])
                h2 = abuf([32, N], bf16)
                layer("sa1", 1, [lambda sl: h1[:, sl]], N,
                      lambda mq, sl: h2[:, sl])
                h3 = abuf([64, N], bf16)
                layer("sa1", 2, [lambda sl: h2[:, sl]], N,
                      lambda mq, sl: h3[:, sl])
                l1fb = small.tile([64, S1], bf16, tag="l1fb")
                nc.vector.tensor_reduce(
                    l1fb[:], h3[:].rearrange("p (s k) -> p s k", k=K1),
                    axis=AX, op=ALU.max)
                l1f = small.tile([64, S1], fp32, tag="l1f")
                nc.vector.tensor_copy(l1f[:], l1fb[:])
                dump("d_l1f", l1f[:], c)

                # ======== SA2 ========
                i2t = small.tile([64, 4, G2 // 64], i16, tag="i2")
                nc.sync.dma_start(i2t[:],
                                  idx2_d[c].rearrange("h p n -> p h n"))
                h1 = abuf([64, G2], bf16)
                blks2, bt2, _ = wt["sa2"][0]
                for hf in range(4):
                    off = hf * (G2 // 4)
                    g2f = g2fp.tile([67, G2 // 4], fp32, tag="g2f")
                    nc.gpsimd.ap_gather(
                        g2f[0:64, :].rearrange("p (n d) -> p n d", d=1),
                        l1f[:].rearrange("p (n d) -> p n d", d=1),
                        i2t[:, hf, :], channels=64, num_elems=S1, d=1,
                        num_idxs=G2 // 4)
                    nc.sync.dma_start(g2f[64:67, :],
                                      g2c_d[c, :, off:off + G2 // 4])
                    if hf == 0:
                        dump("d_g2f", g2f[0:64, :], c)
                    for cb in range(2):
                        ps = pbig.tile([128, CNB], fp32, tag="ps")
                        for sub in range(2):
                            s0 = cb * CNB + sub * CNK
                            nc.tensor.matmul(
                                ps[:64, sub * CNK:(sub + 1) * CNK],
                                blks2[0][:], g2f[:, s0:s0 + CNK],
                                start=True, stop=True)
                        act_store(ps[:64, :], h1[:, off + cb * CNB:
                                                 off + (cb + 1) * CNB],
                                  bt2[:64, :], cb, ACTF.Relu, 0.0)
                h2 = abuf([64, G2], bf16)
                layer("sa2", 1, [lambda sl: h1[:, sl]], G2,
                      lambda mq, sl: h2[:, sl])
                h3 = abuf([128, G2], bf16)
                layer("sa2", 2, [lambda sl: h2[:, sl]], G2,
                      lambda mq, sl: h3[:, sl])
                l2fb = small.tile([128, S2], bf16, tag="l2fb")
                nc.vector.tensor_reduce(
                    l2fb[:], h3[:].rearrange("p (s k) -> p s k", k=K2),
                    axis=AX, op=ALU.max)
                l2f = small.tile([128, S2], fp32, tag="l2f")
                nc.vector.tensor_copy(l2f[:], l2fb[:])
                dump("d_l2f", l2f[:], c)

                # ======== SA3 ========
                i3t = small.tile([128, 2, G3 // 32], i16, tag="i3")
                nc.sync.dma_start(i3t[:],
                                  idx3_d[c].rearrange("h p n -> p h n"))
                g3ct = small.tile([3, G3], bf16, tag="g3c")
                nc.sync.dma_start(g3ct[:], g3c_d[c])
                h1 = abuf([128, G3], bf16)
                blks3, bt3, _ = wt["sa3"][0]
                for hf in range(2):
                    off = hf * (G3 // 2)
                    g3f = g2fp.tile([128, G3 // 2], fp32, tag="g3f",
                                    name="g3f")
                    nc.gpsimd.ap_gather(
                        g3f[:].rearrange("p (n d) -> p n d", d=1),
                        l2f[:].rearrange("p (n d) -> p n d", d=1),
                        i3t[:, hf, :], channels=128, num_elems=S2, d=1,
                        num_idxs=G3 // 2)
                    for cb in range(G3 // 2 // CNB):
                        ps = pbig.tile([128, CNB], fp32, tag="ps")
                        for sub in range(2):
                            s0 = cb * CNB + sub * CNK
                            pslc = ps[:, sub * CNK:(sub + 1) * CNK]
                            nc.tensor.matmul(pslc, blks3[0][:],
                                             g3f[:, s0:s0 + CNK],
                                             start=True, stop=False)
                            nc.tensor.matmul(pslc, blks3[1][:],
                                             g3ct[:, off + s0:off + s0 + CNK],
                                             start=False, stop=True)
                        act_store(ps[:, :],
                                  h1[:, off + cb * CNB:off + (cb + 1) * CNB],
                                  bt3[:, :], cb, ACTF.Relu, 0.0)
                h2 = abuf([128, G3], bf16)
                layer("sa3", 1, [lambda sl: h1[:, sl]], G3,
                      lambda mq, sl: h2[:, sl])
                h3 = abuf([128, 2, G3], bf16)
                layer("sa3", 2, [lambda sl: h2[:, sl]], G3,
                      lambda mq, sl: h3[:, mq, sl])
                l3f = small.tile([128, 2, S3], bf16, tag="l3f")
                for q in range(2):
                    nc.vector.tensor_reduce(
                        l3f[:, q, :],
                        h3[:, q, :].rearrange("p (s k) -> p s k", k=K3),
                        axis=AX, op=ALU.max)
                dump("d_l3f", l3f[:], c)

                # ======== SA4 ========
                l3xt = small.tile([3, S3], bf16, tag="l3x")
                nc.sync.dma_start(l3xt[:], l3x_d[c])
                h1 = small.tile([128, 2, S3], bf16, tag="s4h1")
                layer("sa4", 0, [lambda sl: l3xt[:, sl],
                                 lambda sl: l3f[:, 0, sl],
                                 lambda sl: l3f[:, 1, sl]], S3,
                      lambda mq, sl: h1[:, mq, sl])
                h2 = small.tile([128, 2, S3], bf16, tag="s4h2")
                layer("sa4", 1, [lambda sl: h1[:, 0, sl],
                                 lambda sl: h1[:, 1, sl]], S3,
                      lambda mq, sl: h2[:, mq, sl])
                h4 = small.tile([128, 4, S3], bf16, tag="s4h3")
                layer("sa4", 2, [lambda sl: h2[:, 0, sl],
                                 lambda sl: h2[:, 1, sl]], S3,
                      lambda mq, sl: h4[:, mq, sl])
                l4f = small.tile([128, 4, 1], fp32, tag="l4f")
                for q in range(4):
                    nc.vector.tensor_reduce(
                        l4f[:, q, :], h4[:, q, :], axis=AX, op=ALU.max)
                dump("d_l4f", l4f[:], c)

                # ======== FP4 ========
                b4 = small.tile([128, 4, S3], bf16, tag="b4")
                for q in range(4):
                    nc.vector.tensor_scalar_add(
                        b4[:, q, :], zbf[:, :S3], l4f[:, q, :])
                h1 = small.tile([128, 2, S3], bf16, tag="f4h1")
                layer("fp4", 0,
                      [lambda sl, q=q: b4[:, q, sl] for q in range(4)]
                      + [lambda sl: l3f[:, 0, sl], lambda sl: l3f[:, 1, sl]],
                      S3, lambda mq, sl: h1[:, mq, sl])
                l3fn = small.tile([128, 2, S3], bf16, tag="l3fn")
                layer("fp4", 1, [lambda sl: h1[:, 0, sl],
                                 lambda sl: h1[:, 1, sl]], S3,
                      lambda mq, sl: l3fn[:, mq, sl])
                dump("d_l3fn", l3fn[:], c)

                # ======== FP3 ========
                wi3t = wip.tile([S3, S2], bf16, tag="wi3")
                nc.sync.dma_start(wi3t[:], wi3_d[c])
                l3fT = small.tile([S3, 256], bf16, tag="l3fT")
                for q in range(2):
                    pt = psml.tile([128, CNK], bf16, tag="psm")
                    nc.tensor.transpose(pt[:S3, :128], l3fn[:, q, :],
                                        identb[:])
                    nc.scalar.activation(l3fT[:, q * 128:(q + 1) * 128],
                                         pt[:S3, :128], ACTF.Copy)
                it3 = small.tile([128, 2, S2], bf16, tag="it3")
                for q in range(2):
                    ps = psml.tile([128, CNK], fp32, tag="psm")
                    nc.tensor.matmul(ps[:, :S2],
                                     l3fT[:, q * 128:(q + 1) * 128],
                                     wi3t[:], start=True, stop=True)
                    nc.scalar.activation(it3[:, q, :], ps[:, :S2], ACTF.Copy)
                h1 = small.tile([128, 2, S2], bf16, tag="f3h1")
                layer("fp3", 0, [lambda sl: it3[:, 0, sl],
                                 lambda sl: it3[:, 1, sl],
                                 lambda sl: l2f[:, sl]], S2,
                      lambda mq, sl: h1[:, mq, sl])
                l2fn = small.tile([128, 2, S2], bf16, tag="l2fn")
                layer("fp3", 1, [lambda sl: h1[:, 0, sl],
                                 lambda sl: h1[:, 1, sl]], S2,
                      lambda mq, sl: l2fn[:, mq, sl])
                dump("d_l2fn", l2fn[:], c)

                # ======== FP2 ========
                wi2t = wip.tile([S2, S1], bf16, tag="wi2")
                nc.sync.dma_start(wi2t[:], wi2_d[c])
                l2fT = small.tile([S2, 256], bf16, tag="l2fT")
                for q in range(2):
                    pt = psml.tile([128, CNK], bf16, tag="psm")
                    nc.tensor.transpose(pt[:S2, :128], l2fn[:, q, :],
                                        identb[:])
                    nc.scalar.activation(l2fT[:, q * 128:(q + 1) * 128],
                                         pt[:S2, :128], ACTF.Copy)
                it2 = small.tile([128, 2, S1], bf16, tag="it2")
                for q in range(2):
                    ps = psml.tile([128, CNK], fp32, tag="psm")
                    nc.tensor.matmul(ps[:, :S1],
                                     l2fT[:, q * 128:(q + 1) * 128],
                                     wi2t[:], start=True, stop=True)
                    nc.scalar.activation(it2[:, q, :], ps[:, :S1], ACTF.Copy)
                h1 = small.tile([128, 2, S1], bf16, tag="f2h1")
                layer("fp2", 0, [lambda sl: it2[:, 0, sl],
                                 lambda sl: it2[:, 1, sl],
                                 lambda sl: l1f[:, sl]], S1,
                      lambda mq, sl: h1[:, mq, sl])
                l1fn = small.tile([128, S1], bf16, tag="l1fn")
                layer("fp2", 1, [lambda sl: h1[:, 0, sl],
                                 lambda sl: h1[:, 1, sl]], S1,
                      lambda mq, sl: l1fn[:, sl])
                dump("d_l1fn", l1fn[:], c)

                # ======== FP1 (interp matmul folded into layer 0) ========
                blks, b0t, _ = wt["fp1"][0]
                w0a, w0b = blks
                psy = psml.tile([128, CNK], fp32, tag="psm")
                nc.tensor.matmul(psy[:, :S1], w0a[:], l1fn[:],
                                 start=True, stop=True)
                ysb = small.tile([128, S1], bf16, tag="ysb")
                nc.scalar.activation(ysb[:], psy[:, :S1], ACTF.Copy)
                yT = small.tile([128, 2, 128], bf16, tag="yT")
                for q in range(2):
                    pt = psml.tile([128, CNK], bf16, tag="psm")
                    nc.tensor.transpose(pt[:, :128],
                                        ysb[:, q * 128:(q + 1) * 128],
                                        identb[:])
                    nc.scalar.activation(yT[:, q, :], pt[:, :128], ACTF.Copy)
                wi1t = wi1p.tile([128, 2, N // 2], bf16, tag="wi1")
                nc.sync.dma_start(wi1t[:], wi1_d[c, :, :, :N // 2])
                wi1u = wi1p.tile([128, 2, N // 2], bf16, tag="wi1b")
                nc.sync.dma_start(wi1u[:], wi1_d[c, :, :, N // 2:])
                uft = gin.tile([6, N], bf16, tag="gbuf")
                nc.sync.dma_start(uft[:], uf_d[c])
                h1 = abuf([128, N], bf16)
                for cb in range(N // CNK):
                    sl = slice(cb * CNK, (cb + 1) * CNK)
                    wsrc = wi1t if cb < 8 else wi1u
                    slw = slice((cb % 8) * CNK, (cb % 8 + 1) * CNK)
                    ps = pbig.tile([128, CNK], fp32, tag="ps")
                    nc.tensor.matmul(ps[:, :], yT[:, 0, :], wsrc[:, 0, slw],
                                     start=True, stop=False)
                    nc.tensor.matmul(ps[:, :], yT[:, 1, :], wsrc[:, 1, slw],
                                     start=False, stop=False)
                    nc.tensor.matmul(ps[:, :], w0b[:], uft[:, sl],
                                     start=False, stop=True)
                    act_store(ps[:, :], h1[:, sl], b0t[:, :], cb,
                              ACTF.Relu, 0.0)
                dump("d_h1fp1", h1[:], c)
                feat = abuf([128, N], bf16)
                layer("fp1", 1, [lambda sl: h1[:, sl]], N,
                      lambda mq, sl: feat[:, sl])
                dump("d_feat", feat[:], c)

                # ======== heads ========
                t1 = abuf([128, N], bf16)
                layer("head1", 0, [lambda sl: feat[:, sl]], N,
                      lambda mq, sl: t1[:, sl], func="lrelu")
                dump("d_t1", t1[:], c)
                t2 = abuf([128, N], bf16)
                layer("head2", 0, [lambda sl: t1[:, sl]], N,
                      lambda mq, sl: t2[:, sl], func="lrelu")
                w3 = wt["head3"][0][0][0]
                for cb in range(N // CNB):
                    bsl = slice(cb * CNB, (cb + 1) * CNB)
                    ps = pbig.tile([128, CNB], fp32, tag="ps")
                    for sub in range(2):
                        s0 = cb * CNB + sub * CNK
                        nc.tensor.matmul(ps[:, sub * CNK:(sub + 1) * CNK],
                                         w3[:, 0:128], t2[:, s0:s0 + CNK],
                                         start=True, stop=True)
                    osb = outp.tile([128, CNB], fp32, tag="osb")
                    if cb % 2 == 0:
                        nc.scalar.activation(osb[:], ps[:, :], ACTF.Copy)
                    else:
                        nc.vector.tensor_copy(osb[:], ps[:, :])
                    nc.sync.dma_start(out_d[c, 0:128, bsl], osb[:])
                    for sub in range(2):
                        s0 = cb * CNB + sub * CNK
                        ps2 = psml.tile([128, CNK], fp32, tag="psm",
                                        name="ps2")
                        nc.tensor.matmul(ps2[:7, :], w3[:, 128:135],
                                         t2[:, s0:s0 + CNK],
                                         start=True, stop=True)
                        osb2 = outp.tile([7, CNK], fp32, tag="osb2",
                                         name="osb2")
                        nc.vector.tensor_copy(osb2[:], ps2[:7, :])
                        nc.sync.dma_start(out_d[c, 128:135, s0:s0 + CNK],
                                          osb2[:])

    nc.compile()
    return nc


def _get_built():
    global _BUILT
    if _BUILT is None:
        _BUILT = _build()
    return _BUILT


def run_device(per_core, trace=False, tmpdir=None):
    from concourse.bass_utils import run_bass_kernel_spmd

    nc = _get_built()
    res = run_bass_kernel_spmd(nc, per_core, core_ids=list(range(NCORES)),
                               trace=trace, tmpdir=tmpdir)
    out = np.concatenate([r["out"] for r in res.results], 0)
    return out, res


def kernel(xyz, points, params):
    xyz = np.asarray(xyz, np.float32)
    points = np.asarray(points, np.float32)
    per_core = _prep_host(xyz, points, params)
    out, _ = run_device(per_core)
    bat = out.transpose(0, 2, 1).reshape(B, N, CH, T)
    return bat[:, :, :2, :], bat[:, :, 2:-20, :], bat[:, :, -20:, :]


# revision 36
# speedup vs baseline: 1.0548x; 1.0428x over previous
"""PointNet++ backbone (nn_BackbonePointNet2) on 8 Trainium2 NeuronCores.

Sharding: data-parallel over batch. B=32 clouds -> 4 clouds per core.
Host precomputes index structure (FPS order, ball-query neighbor lists,
3-NN interpolation indices/weights) in numpy; the device kernel does all
dense compute: grouping gathers (gpsimd ap_gather), every shared-MLP layer
(TensorE matmuls, fp32 PSUM accum), fused BN+ReLU (ACT/DVE), max-pools
(DVE reduce), FP interpolation matmuls, heads with fused leaky-relu.

Activations on SBUF are logically [C, n] with C on partitions; C > 128 is
stored as [128, q, n] (q = C // 128).
"""

import numpy as np
import ml_dtypes

B, N = 32, 8192
T, CH = 5, 27
NCORES = 8
BPC = B // NCORES  # clouds per core

S1, K1, R1 = 256, 32, 0.05
S2, K2, R2 = 128, 64, 0.1
S3, K3, R3 = 32, 128, 0.2
G2 = S2 * K2   # 8192
G3 = S3 * K3   # 4096

BF16 = ml_dtypes.bfloat16

# K-block split per layer: (rows, bf16?) — order must match kernel rhs order.
_SPLITS = {
    "sa1": [[(6, 1)], [(32, 1)], [(32, 1)]],
    "sa2": [[(67, 0)], [(64, 1)], [(64, 1)]],
    "sa3": [[(128, 0), (3, 1)], [(128, 1)], [(128, 1)]],
    "sa4": [[(3, 1), (128, 1), (128, 1)], [(128, 1), (128, 1)],
            [(128, 1), (128, 1)]],
    "fp4": [[(128, 1)] * 6, [(128, 1), (128, 1)]],
    "fp3": [[(128, 1), (128, 1), (128, 0)], [(128, 1), (128, 1)]],
    "fp2": [[(128, 1), (128, 1), (64, 0)], [(128, 1), (128, 1)]],
    "fp1": [[(128, 1), (6, 1)], [(128, 1)]],
    "head1": [[(128, 1)]],
    "head2": [[(128, 1)]],
    "head3": [[(128, 1)]],
}
_COUT = {"sa1": [32, 32, 64], "sa2": [64, 64, 128], "sa3": [128, 128, 256],
         "sa4": [256, 256, 512], "fp4": [256, 256], "fp3": [256, 256],
         "fp2": [256, 128], "fp1": [128, 128], "head1": [128],
         "head2": [128], "head3": [CH * T]}

# ----------------------------------------------------------------- host math


def _fps(xyz, npoint):
    Bb, Nn, _ = xyz.shape
    dist = np.full((Bb, Nn), 1e10, np.float32)
    idxs = np.zeros((Bb, npoint), np.int64)
    last = np.zeros((Bb,), np.int64)
    ar = np.arange(Bb)
    for s in range(1, npoint):
        p = xyz[ar, last]
        d = ((xyz - p[:, None, :]) ** 2).sum(-1)
        dist = np.minimum(dist, d)
        last = dist.argmax(-1)
        idxs[:, s] = last
    return idxs


def _ball(new_xyz, xyz, r, k):
    d2 = ((new_xyz[:, None, :] - xyz[None, :, :]) ** 2).sum(-1)
    Nn = xyz.shape[0]
    keyv = np.where(d2 < r * r, np.arange(Nn)[None, :], Nn)
    part = np.partition(keyv, k - 1, axis=-1)[:, :k] if k < Nn else keyv
    order = np.sort(part, -1)[:, :k]
    first = order[:, :1]
    idx = np.where(order < Nn, order, np.where(first < Nn, first, 0))
    return idx.astype(np.int64)


def _interp_mat(unknown, known, n, m):
    d2 = ((unknown[:, None, :] - known[None, :, :]) ** 2).sum(-1)
    ii = np.argsort(d2, axis=-1, kind="stable")[:, :3]
    dsel = np.take_along_axis(d2, ii, -1).astype(np.float32)
    dist = np.sqrt(np.maximum(dsel, 0.0))
    w = 1.0 / (dist + 1e-8)
    w = (w / w.sum(-1, keepdims=True)).astype(np.float32)
    W = np.zeros((m, n), np.float32)
    ar = np.arange(n)
    for k in range(3):
        W[ii[:, k], ar] += w[:, k]
    return W


def _wrap16(idx, parts):
    n = idx.shape[0]
    w = idx.reshape(n // 16, 16).T.astype(np.int16)
    return np.tile(w, (parts // 16, 1))


def _prep_host(xyz, points, params):
    fidx1 = _fps(xyz, S1)
    l1x = np.take_along_axis(xyz, fidx1[..., None], axis=1)
    fidx2 = _fps(l1x, S2)
    l2x = np.take_along_axis(l1x, fidx2[..., None], axis=1)
    fidx3 = _fps(l2x, S3)
    l3x = np.take_along_axis(l2x, fidx3[..., None], axis=1)

    ptsT = np.transpose(points, (0, 2, 1))

    g1 = np.empty((B, 6, S1 * K1), np.float32)
    g2c = np.empty((B, 3, G2), np.float32)
    g3c = np.empty((B, 3, G3), np.float32)
    idx2w = np.empty((B, 4, 64, G2 // 64), np.int16)   # four quarters
    idx3w = np.empty((B, 2, 128, G3 // 32), np.int16)
    wi3 = np.empty((B, S3, S2), np.float32)
    wi2 = np.empty((B, S2, S1), np.float32)
    wi1 = np.empty((B, S1, N), np.float32)
    for b in range(B):
        i1 = _ball(l1x[b], xyz[b], R1, K1)
        gx = xyz[b][i1] - l1x[b][:, None, :]
        gn = ptsT[b][i1]
        g1[b] = np.concatenate([gx, gn], -1).transpose(2, 0, 1).reshape(6, -1)

        i2 = _ball(l2x[b], l1x[b], R2, K2)
        i2f = i2.reshape(-1)
        for qq in range(4):
            idx2w[b, qq] = _wrap16(i2f[qq * G2 // 4:(qq + 1) * G2 // 4], 64)
        g2c[b] = (l1x[b][i2] - l2x[b][:, None, :]).transpose(2, 0, 1).reshape(3, -1)

        i3 = _ball(l3x[b], l2x[b], R3, K3)
        g3c[b] = (l2x[b][i3] - l3x[b][:, None, :]).transpose(2, 0, 1).reshape(3, -1)
        i3f = i3.reshape(-1)
        idx3w[b, 0] = _wrap16(i3f[:G3 // 2], 128)
        idx3w[b, 1] = _wrap16(i3f[G3 // 2:], 128)

        wi3[b] = _interp_mat(l2x[b], l3x[b], S2, S3)
        wi2[b] = _interp_mat(l1x[b], l2x[b], S1, S2)
        wi1[b] = _interp_mat(xyz[b], l1x[b], N, S1)

    uf = np.concatenate([np.transpose(xyz, (0, 2, 1)), points], 1)

    wblobs = {}
    for nm, layers in _SPLITS.items():
        for li, blocks in enumerate(layers):
            W, g, bb = params[nm][li]
            W = np.asarray(W, np.float32) * np.asarray(g, np.float32)[:, None]
            WT = W.T.copy()
            if nm in ("sa2", "sa3") and li == 0:
                WT = np.concatenate([WT[3:], WT[:3]], 0)  # [feat; coord]
            r0 = 0
            for ki, (kb, isbf) in enumerate(blocks):
                blk = WT[r0:r0 + kb]
                wblobs[f"W_{nm}_{li}_{ki}"] = (
                    blk.astype(BF16) if isbf else blk.astype(np.float32))
                r0 += kb
            assert r0 == WT.shape[0], (nm, li, r0, WT.shape)
            bb = np.asarray(bb, np.float32)
            Cout = bb.shape[0]
            if Cout % 128 == 0 and Cout > 128:
                bmat = bb.reshape(Cout // 128, 128).T.copy()
            elif Cout < 128:
                bmat = np.tile(bb, 128 // Cout)[:128, None].copy()
            else:
                bmat = bb[:, None].copy()
            wblobs[f"b_{nm}_{li}"] = bmat

    per_core = []
    for c in range(NCORES):
        sl = slice(c * BPC, (c + 1) * BPC)
        m = {
            "g1": g1[sl].astype(BF16),
            "g2c": g2c[sl].astype(np.float32),
            "g3c": g3c[sl].astype(BF16),
            "idx2": idx2w[sl],
            "idx3": idx3w[sl],
            "l3x": l3x[sl].transpose(0, 2, 1).astype(BF16).copy(),
            "wi3": wi3[sl].astype(BF16),
            "wi2": wi2[sl].astype(BF16),
            "wi1": wi1[sl].reshape(BPC, 2, 128, N).astype(BF16),
            "uf": uf[sl].astype(BF16),
        }
        m.update(wblobs)
        per_core.append(m)
    return per_core


# ------------------------------------------------------------- device kernel

_BUILT = None
CNK = 512
CNB = 1024


def _build():
    import contextlib
    import concourse.mybir as mybir
    import concourse.tile as tile
    from concourse import bacc
    from concourse.masks import make_identity

    fp32 = mybir.dt.float32
    bf16 = mybir.dt.bfloat16
    i16 = mybir.dt.int16
    AX = mybir.AxisListType.X
    ALU = mybir.AluOpType
    ACTF = mybir.ActivationFunctionType

    nc = bacc.Bacc("TRN2", target_bir_lowering=False, debug=False)

    def din(name, shape, dt):
        return nc.dram_tensor(name, shape, dt, kind="ExternalInput").ap()

    g1_d = din("g1", [BPC, 6, S1 * K1], bf16)
    g2c_d = din("g2c", [BPC, 3, G2], fp32)
    g3c_d = din("g3c", [BPC, 3, G3], bf16)
    idx2_d = din("idx2", [BPC, 4, 64, G2 // 64], i16)
    idx3_d = din("idx3", [BPC, 2, 128, G3 // 32], i16)
    l3x_d = din("l3x", [BPC, 3, S3], bf16)
    wi3_d = din("wi3", [BPC, S3, S2], bf16)
    wi2_d = din("wi2", [BPC, S2, S1], bf16)
    wi1_d = din("wi1", [BPC, 2, 128, N], bf16)
    uf_d = din("uf", [BPC, 6, N], bf16)
    out_d = nc.dram_tensor("out", [BPC, CH * T, N], fp32,
                           kind="ExternalOutput").ap()
    import os
    dbg = os.environ.get("BASSDBG", "0") == "1"
    dbg_d = {}
    if dbg:
        for nm_, shp, dt_ in [("d_l1f", [64, S1], fp32),
                              ("d_l2f", [128, S2], fp32),
                              ("d_l3f", [128, 2, S3], bf16),
                              ("d_l4f", [128, 4, 1], fp32),
                              ("d_l3fn", [128, 2, S3], bf16),
                              ("d_l2fn", [128, 2, S2], bf16),
                              ("d_l1fn", [128, S1], bf16),
                              ("d_h1fp1", [128, N], bf16),
                              ("d_feat", [128, N], bf16),
                              ("d_g2f", [64, G2 // 2], fp32),
                              ("d_t1", [128, N], bf16)]:
            dbg_d[nm_] = nc.dram_tensor(nm_, shp, dt_,
                                        kind="ExternalOutput").ap()

    wdecl = {}
    for nm, layers in _SPLITS.items():
        wdecl[nm] = []
        for li, blocks in enumerate(layers):
            Cout = _COUT[nm][li]
            blks = [din(f"W_{nm}_{li}_{ki}", [kb, Cout],
                        bf16 if isbf else fp32)
                    for ki, (kb, isbf) in enumerate(blocks)]
            bshape = ([128, Cout // 128] if (Cout % 128 == 0 and Cout > 128)
                      else [min(Cout, 128), 1])
            wdecl[nm].append((blks, din(f"b_{nm}_{li}", bshape, fp32), Cout))

    with tile.TileContext(nc) as tc:
        with contextlib.ExitStack() as ctx:
            wpool = ctx.enter_context(tc.tile_pool(name="w", bufs=1))
            cpool = ctx.enter_context(tc.tile_pool(name="const", bufs=1))
            gin = ctx.enter_context(tc.tile_pool(name="gin", bufs=1))
            act16 = ctx.enter_context(tc.tile_pool(name="act16", bufs=2))
            g2fp = ctx.enter_context(tc.tile_pool(name="g2f", bufs=1))
            small = ctx.enter_context(tc.tile_pool(name="small", bufs=2))
            wip = ctx.enter_context(tc.tile_pool(name="wi", bufs=2))
            wi1p = ctx.enter_context(tc.tile_pool(name="wi1", bufs=1))
            outp = ctx.enter_context(tc.tile_pool(name="outp", bufs=1))
            pbig = ctx.enter_context(tc.tile_pool(name="pbig", bufs=3,
                                                  space="PSUM"))
            psml = ctx.enter_context(tc.tile_pool(name="psml", bufs=2,
                                                  space="PSUM"))

            identb = cpool.tile([128, 128], bf16, tag="idbf")
            make_identity(nc, identb[:])
            zbf = cpool.tile([128, CNB], bf16, tag="zbf")
            nc.vector.memset(zbf[:], 0.0)

            wt = {}
            for nm, layers in wdecl.items():
                wt[nm] = []
                for li, (blks, bd, Cout) in enumerate(layers):
                    tl = []
                    for ki, wd_ in enumerate(blks):
                        t = wpool.tile(wd_.shape, wd_.dtype,
                                       tag=f"W{nm}{li}{ki}")
                        nc.sync.dma_start(t[:], wd_)
                        tl.append(t)
                    bt = wpool.tile(bd.shape, fp32, tag=f"b{nm}{li}")
                    nc.sync.dma_start(bt[:], bd)
                    wt[nm].append((tl, bt, Cout))

            # two rotating activation buffers (16KB slots)
            _rot = [0]

            def abuf(shape, dt):
                _rot[0] ^= 1
                tg = "actA" if _rot[0] else "actB"
                return act16.tile(shape, dt, tag=tg, name=tg)


            def dump(nm_, ap, c_):
                if dbg and c_ == 0:
                    nc.sync.dma_start(dbg_d[nm_], ap)
            def act_store(ps_ap, out_ap, bias_ap, alt, func, alpha):
                if func == ACTF.Lrelu:
                    nc.scalar.activation(out_ap, ps_ap, ACTF.Prelu,
                                         bias=bias_ap, alpha=float(alpha))
                elif func == ACTF.Relu and alt % 2 == 1 and out_ap.dtype == bf16:
                    nc.vector.scalar_tensor_tensor(
                        out_ap, ps_ap, bias_ap,
                        zbf[:ps_ap.shape[0], :ps_ap.shape[-1]],
                        op0=ALU.add, op1=ALU.max)
                elif func is None:
                    nc.scalar.activation(out_ap, ps_ap, ACTF.Copy)
                else:
                    nc.scalar.activation(out_ap, ps_ap, func,
                                         bias=bias_ap, alpha=alpha)

            def layer(nm, li, rhs_fns, n, out_fn, func="relu"):
                blks, bt, Cout = wt[nm][li]
                fn = {"relu": ACTF.Relu, "lrelu": ACTF.Lrelu,
                      "none": None}[func]
                alpha = 0.2 if func == "lrelu" else 0.0
                alt = 0
                for mq in range((Cout + 127) // 128):
                    mw = min(128, Cout - mq * 128)
                    for cb in range((n + CNB - 1) // CNB):
                        c0 = cb * CNB
                        cw = min(CNB, n - c0)
                        ps = pbig.tile([128, CNB], fp32, tag="ps")
                        for sub in range(0, cw, CNK):
                            sw = min(CNK, cw - sub)
                            sl = slice(c0 + sub, c0 + sub + sw)
                            for ki, rf in enumerate(rhs_fns):
                                nc.tensor.matmul(
                                    ps[:mw, sub:sub + sw],
                                    blks[ki][:, mq * 128:mq * 128 + mw],
                                    rf(sl), start=(ki == 0),
                                    stop=(ki == len(rhs_fns) - 1))
                        bap = (bt[:mw, mq:mq + 1] if bt.shape[1] > 1
                               else bt[:mw, :])
                        act_store(ps[:mw, :cw], out_fn(mq, slice(c0, c0 + cw)),
                                  bap, alt, fn, alpha)
                        alt += 1

            for c in range(BPC):
                # ======== SA1 ========
                g1t = gin.tile([6, S1 * K1], bf16, tag="gbuf")
                nc.sync.dma_start(g1t[:], g1_d[c])
                h1 = abuf([32, N], bf16)
                layer("sa1", 0, [lambda sl: g1t[:, sl]], N,
                      lambda mq, sl: h1[:, sl])
                h2 = abuf([32, N], bf16)
                layer("sa1", 1, [lambda sl: h1[:, sl]], N,
                      lambda mq, sl: h2[:, sl])
                h3 = abuf([64, N], bf16)
                layer("sa1", 2, [lambda sl: h2[:, sl]], N,
                      lambda mq, sl: h3[:, sl])
                l1fb = small.tile([64, S1], bf16, tag="l1fb")
                nc.vector.tensor_reduce(
                    l1fb[:], h3[:].rearrange("p (s k) -> p s k", k=K1),
                    axis=AX, op=ALU.max)
                l1f = small.tile([64, S1], fp32, tag="l1f")
                nc.vector.tensor_copy(l1f[:], l1fb[:])
                dump("d_l1f", l1f[:], c)

                # ======== SA2 ========
                i2t = small.tile([64, 4, G2 // 64], i16, tag="i2")
                nc.sync.dma_start(i2t[:],
                                  idx2_d[c].rearrange("h p n -> p h n"))
                h1 = abuf([64, G2], bf16)
                blks2, bt2, _ = wt["sa2"][0]
                for hf in range(4):
                    off = hf * (G2 // 4)
                    g2f = g2fp.tile([67, G2 // 4], fp32, tag="g2f",
                                    bufs=2)
                    nc.gpsimd.ap_gather(
                        g2f[0:64, :].rearrange("p (n d) -> p n d", d=1),
                        l1f[:].rearrange("p (n d) -> p n d", d=1),
                        i2t[:, hf, :], channels=64, num_elems=S1, d=1,
                        num_idxs=G2 // 4)
                    nc.sync.dma_start(g2f[64:67, :],
                                      g2c_d[c, :, off:off + G2 // 4])
                    if hf == 0:
                        dump("d_g2f", g2f[0:64, :], c)
                    for cb in range(2):
                        ps = pbig.tile([128, CNB], fp32, tag="ps")
                        for sub in range(2):
                            s0 = cb * CNB + sub * CNK
                            nc.tensor.matmul(
                                ps[:64, sub * CNK:(sub + 1) * CNK],
                                blks2[0][:], g2f[:, s0:s0 + CNK],
                                start=True, stop=True)
                        act_store(ps[:64, :], h1[:, off + cb * CNB:
                                                 off + (cb + 1) * CNB],
                                  bt2[:64, :], cb, ACTF.Relu, 0.0)
                h2 = abuf([64, G2], bf16)
                layer("sa2", 1, [lambda sl: h1[:, sl]], G2,
                      lambda mq, sl: h2[:, sl])
                h3 = abuf([128, G2], bf16)
                layer("sa2", 2, [lambda sl: h2[:, sl]], G2,
                      lambda mq, sl: h3[:, sl])
                l2fb = small.tile([128, S2], bf16, tag="l2fb")
                nc.vector.tensor_reduce(
                    l2fb[:], h3[:].rearrange("p (s k) -> p s k", k=K2),
                    axis=AX, op=ALU.max)
                l2f = small.tile([128, S2], fp32, tag="l2f")
                nc.vector.tensor_copy(l2f[:], l2fb[:])
                dump("d_l2f", l2f[:], c)

                # ======== SA3 ========
                i3t = small.tile([128, 2, G3 // 32], i16, tag="i3")
                nc.sync.dma_start(i3t[:],
                                  idx3_d[c].rearrange("h p n -> p h n"))
                g3ct = small.tile([3, G3], bf16, tag="g3c")
                nc.sync.dma_start(g3ct[:], g3c_d[c])
                h1 = abuf([128, G3], bf16)
                blks3, bt3, _ = wt["sa3"][0]
                for hf in range(2):
                    off = hf * (G3 // 2)
                    g3f = g2fp.tile([128, G3 // 2], fp32, tag="g3f",
                                    name="g3f")
                    nc.gpsimd.ap_gather(
                        g3f[:].rearrange("p (n d) -> p n d", d=1),
                        l2f[:].rearrange("p (n d) -> p n d", d=1),
                        i3t[:, hf, :], channels=128, num_elems=S2, d=1,
                        num_idxs=G3 // 2)
                    for cb in range(G3 // 2 // CNB):
                        ps = pbig.tile([128, CNB], fp32, tag="ps")
                        for sub in range(2):
                            s0 = cb * CNB + sub * CNK
                            pslc = ps[:, sub * CNK:(sub + 1) * CNK]
                            nc.tensor.matmul(pslc, blks3[0][:],
                                             g3f[:, s0:s0 + CNK],
                                             start=True, stop=False)
                            nc.tensor.matmul(pslc, blks3[1][:],
                                             g3ct[:, off + s0:off + s0 + CNK],
                                             start=False, stop=True)
                        act_store(ps[:, :],
                                  h1[:, off + cb * CNB:off + (cb + 1) * CNB],
                                  bt3[:, :], cb, ACTF.Relu, 0.0)
                h2 = abuf([128, G3], bf16)
                layer("sa3", 1, [lambda sl: h1[:, sl]], G3,
                      lambda mq, sl: h2[:, sl])
                h3 = abuf([128, 2, G3], bf16)
                layer("sa3", 2, [lambda sl: h2[:, sl]], G3,
                      lambda mq, sl: h3[:, mq, sl])
                l3f = small.tile([128, 2, S3], bf16, tag="l3f")
                for q in range(2):
                    nc.vector.tensor_reduce(
                        l3f[:, q, :],
                        h3[:, q, :].rearrange("p (s k) -> p s k", k=K3),
                        axis=AX, op=ALU.max)
                dump("d_l3f", l3f[:], c)

                # ======== SA4 ========
                l3xt = small.tile([3, S3], bf16, tag="l3x")
                nc.sync.dma_start(l3xt[:], l3x_d[c])
                h1 = small.tile([128, 2, S3], bf16, tag="s4h1")
                layer("sa4", 0, [lambda sl: l3xt[:, sl],
                                 lambda sl: l3f[:, 0, sl],
                                 lambda sl: l3f[:, 1, sl]], S3,
                      lambda mq, sl: h1[:, mq, sl])
                h2 = small.tile([128, 2, S3], bf16, tag="s4h2")
                layer("sa4", 1, [lambda sl: h1[:, 0, sl],
                                 lambda sl: h1[:, 1, sl]], S3,
                      lambda mq, sl: h2[:, mq, sl])
                h4 = small.tile([128, 4, S3], bf16, tag="s4h3")
                layer("sa4", 2, [lambda sl: h2[:, 0, sl],
                                 lambda sl: h2[:, 1, sl]], S3,
                      lambda mq, sl: h4[:, mq, sl])
                l4f = small.tile([128, 4, 1], fp32, tag="l4f")
                for q in range(4):
                    nc.vector.tensor_reduce(
                        l4f[:, q, :], h4[:, q, :], axis=AX, op=ALU.max)
                dump("d_l4f", l4f[:], c)

                # ======== FP4 ========
                b4 = small.tile([128, 4, S3], bf16, tag="b4")
                for q in range(4):
                    nc.vector.tensor_scalar_add(
                        b4[:, q, :], zbf[:, :S3], l4f[:, q, :])
                h1 = small.tile([128, 2, S3], bf16, tag="f4h1")
                layer("fp4", 0,
                      [lambda sl, q=q: b4[:, q, sl] for q in range(4)]
                      + [lambda sl: l3f[:, 0, sl], lambda sl: l3f[:, 1, sl]],
                      S3, lambda mq, sl: h1[:, mq, sl])
                l3fn = small.tile([128, 2, S3], bf16, tag="l3fn")
                layer("fp4", 1, [lambda sl: h1[:, 0, sl],
                                 lambda sl: h1[:, 1, sl]], S3,
                      lambda mq, sl: l3fn[:, mq, sl])
                dump("d_l3fn", l3fn[:], c)

                # ======== FP3 ========
                wi3t = wip.tile([S3, S2], bf16, tag="wi3")
                nc.sync.dma_start(wi3t[:], wi3_d[c])
                l3fT = small.tile([S3, 256], bf16, tag="l3fT")
                for q in range(2):
                    pt = psml.tile([128, CNK], bf16, tag="psm")
                    nc.tensor.transpose(pt[:S3, :128], l3fn[:, q, :],
                                        identb[:])
                    nc.scalar.activation(l3fT[:, q * 128:(q + 1) * 128],
                                         pt[:S3, :128], ACTF.Copy)
                it3 = small.tile([128, 2, S2], bf16, tag="it3")
                for q in range(2):
                    ps = psml.tile([128, CNK], fp32, tag="psm")
                    nc.tensor.matmul(ps[:, :S2],
                                     l3fT[:, q * 128:(q + 1) * 128],
                                     wi3t[:], start=True, stop=True)
                    nc.scalar.activation(it3[:, q, :], ps[:, :S2], ACTF.Copy)
                h1 = small.tile([128, 2, S2], bf16, tag="f3h1")
                layer("fp3", 0, [lambda sl: it3[:, 0, sl],
                                 lambda sl: it3[:, 1, sl],
                                 lambda sl: l2f[:, sl]], S2,
                      lambda mq, sl: h1[:, mq, sl])
                l2fn = small.tile([128, 2, S2], bf16, tag="l2fn")
                layer("fp3", 1, [lambda sl: h1[:, 0, sl],
                                 lambda sl: h1[:, 1, sl]], S2,
                      lambda mq, sl: l2fn[:, mq, sl])
                dump("d_l2fn", l2fn[:], c)

                # ======== FP2 ========
                wi2t = wip.tile([S2, S1], bf16, tag="wi2")
                nc.sync.dma_start(wi2t[:], wi2_d[c])
                l2fT = small.tile([S2, 256], bf16, tag="l2fT")
                for q in range(2):
                    pt = psml.tile([128, CNK], bf16, tag="psm")
                    nc.tensor.transpose(pt[:S2, :128], l2fn[:, q, :],
                                        identb[:])
                    nc.scalar.activation(l2fT[:, q * 128:(q + 1) * 128],
                                         pt[:S2, :128], ACTF.Copy)
                it2 = small.tile([128, 2, S1], bf16, tag="it2")
                for q in range(2):
                    ps = psml.tile([128, CNK], fp32, tag="psm")
                    nc.tensor.matmul(ps[:, :S1],
                                     l2fT[:, q * 128:(q + 1) * 128],
                                     wi2t[:], start=True, stop=True)
                    nc.scalar.activation(it2[:, q, :], ps[:, :S1], ACTF.Copy)
                h1 = small.tile([128, 2, S1], bf16, tag="f2h1")
                layer("fp2", 0, [lambda sl: it2[:, 0, sl],
                                 lambda sl: it2[:, 1, sl],
                                 lambda sl: l1f[:, sl]], S1,
                      lambda mq, sl: h1[:, mq, sl])
                l1fn = small.tile([128, S1], bf16, tag="l1fn")
                layer("fp2", 1, [lambda sl: h1[:, 0, sl],
                                 lambda sl: h1[:, 1, sl]], S1,
                      lambda mq, sl: l1fn[:, sl])
                dump("d_l1fn", l1fn[:], c)

                # ======== FP1 (interp matmul folded into layer 0) ========
                blks, b0t, _ = wt["fp1"][0]
                w0a, w0b = blks
                psy = psml.tile([128, CNK], fp32, tag="psm")
                nc.tensor.matmul(psy[:, :S1], w0a[:], l1fn[:],
                                 start=True, stop=True)
                ysb = small.tile([128, S1], bf16, tag="ysb")
                nc.scalar.activation(ysb[:], psy[:, :S1], ACTF.Copy)
                yT = small.tile([128, 2, 128], bf16, tag="yT")
                for q in range(2):
                    pt = psml.tile([128, CNK], bf16, tag="psm")
                    nc.tensor.transpose(pt[:, :128],
                                        ysb[:, q * 128:(q + 1) * 128],
                                        identb[:])
                    nc.scalar.activation(yT[:, q, :], pt[:, :128], ACTF.Copy)
                wi1t = wi1p.tile([128, 2, N // 2], bf16, tag="wi1")
                nc.sync.dma_start(wi1t[:], wi1_d[c, :, :, :N // 2])
                wi1u = wi1p.tile([128, 2, N // 2], bf16, tag="wi1b")
                nc.sync.dma_start(wi1u[:], wi1_d[c, :, :, N // 2:])
                uft = gin.tile([6, N], bf16, tag="gbuf")
                nc.sync.dma_start(uft[:], uf_d[c])
                h1 = abuf([128, N], bf16)
                for cb in range(N // CNK):
                    sl = slice(cb * CNK, (cb + 1) * CNK)
                    wsrc = wi1t if cb < 8 else wi1u
                    slw = slice((cb % 8) * CNK, (cb % 8 + 1) * CNK)
                    ps = pbig.tile([128, CNK], fp32, tag="ps")
                    nc.tensor.matmul(ps[:, :], yT[:, 0, :], wsrc[:, 0, slw],
                                     start=True, stop=False)
                    nc.tensor.matmul(ps[:, :], yT[:, 1, :], wsrc[:, 1, slw],
                                     start=False, stop=False)
                    nc.tensor.matmul(ps[:, :], w0b[:], uft[:, sl],
                                     start=False, stop=True)
                    act_store(ps[:, :], h1[:, sl], b0t[:, :], cb,
                              ACTF.Relu, 0.0)
                dump("d_h1fp1", h1[:], c)
                feat = abuf([128, N], bf16)
                layer("fp1", 1, [lambda sl: h1[:, sl]], N,
                      lambda mq, sl: feat[:, sl])
                dump("d_feat", feat[:], c)

                # ======== heads ========
                t1 = abuf([128, N], bf16)
                layer("head1", 0, [lambda sl: feat[:, sl]], N,
                      lambda mq, sl: t1[:, sl], func="lrelu")
                dump("d_t1", t1[:], c)
                t2 = abuf([128, N], bf16)
                layer("head2", 0, [lambda sl: t1[:, sl]], N,
                      lambda mq, sl: t2[:, sl], func="lrelu")
                w3 = wt["head3"][0][0][0]
                for cb in range(N // CNB):
                    bsl = slice(cb * CNB, (cb + 1) * CNB)
                    ps = pbig.tile([128, CNB], fp32, tag="ps")
                    for sub in range(2):
                        s0 = cb * CNB + sub * CNK
                        nc.tensor.matmul(ps[:, sub * CNK:(sub + 1) * CNK],
                                         w3[:, 0:128], t2[:, s0:s0 + CNK],
                                         start=True, stop=True)
                    osb = outp.tile([128, CNB], fp32, tag="osb")
                    if cb % 2 == 0:
                        nc.scalar.activation(osb[:], ps[:, :], ACTF.Copy)
                    else:
                        nc.vector.tensor_copy(osb[:], ps[:, :])
                    nc.sync.dma_start(out_d[c, 0:128, bsl], osb[:])
                    for sub in range(2):
                        s0 = cb * CNB + sub * CNK
                        ps2 = psml.tile([128, CNK], fp32, tag="psm",
                                        name="ps2")
                        nc.tensor.matmul(ps2[:7, :], w3[:, 128:135],
                                         t2[:, s0:s0 + CNK],
                                         start=True, stop=True)
                        osb2 = outp.tile([7, CNK], fp32, tag="osb2",
                                         name="osb2")
                        nc.vector.tensor_copy(osb2[:], ps2[:7, :])
                        nc.sync.dma_start(out_d[c, 128:135, s0:s0 + CNK],
                                          osb2[:])

    nc.compile()
    return nc


def _get_built():
    global _BUILT
    if _BUILT is None:
        _BUILT = _build()
    return _BUILT


def run_device(per_core, trace=False, tmpdir=None):
    from concourse.bass_utils import run_bass_kernel_spmd

    nc = _get_built()
    res = run_bass_kernel_spmd(nc, per_core, core_ids=list(range(NCORES)),
                               trace=trace, tmpdir=tmpdir)
    out = np.concatenate([r["out"] for r in res.results], 0)
    return out, res


def kernel(xyz, points, params):
    xyz = np.asarray(xyz, np.float32)
    points = np.asarray(points, np.float32)
    per_core = _prep_host(xyz, points, params)
    out, _ = run_device(per_core)
    bat = out.transpose(0, 2, 1).reshape(B, N, CH, T)
    return bat[:, :, :2, :], bat[:, :, 2:-20, :], bat[:, :, -20:, :]


# revision 38
# speedup vs baseline: 1.0666x; 1.0113x over previous
"""PointNet++ backbone (nn_BackbonePointNet2) on 8 Trainium2 NeuronCores.

Sharding: data-parallel over batch. B=32 clouds -> 4 clouds per core.
Host precomputes index structure (FPS order, ball-query neighbor lists,
3-NN interpolation indices/weights) in numpy; the device kernel does all
dense compute: grouping gathers (gpsimd ap_gather), every shared-MLP layer
(TensorE matmuls, fp32 PSUM accum), fused BN+ReLU (ACT/DVE), max-pools
(DVE reduce), FP interpolation matmuls, heads with fused leaky-relu.

Activations on SBUF are logically [C, n] with C on partitions; C > 128 is
stored as [128, q, n] (q = C // 128).
"""

import numpy as np
import ml_dtypes

B, N = 32, 8192
T, CH = 5, 27
NCORES = 8
BPC = B // NCORES  # clouds per core

S1, K1, R1 = 256, 32, 0.05
S2, K2, R2 = 128, 64, 0.1
S3, K3, R3 = 32, 128, 0.2
G2 = S2 * K2   # 8192
G3 = S3 * K3   # 4096

BF16 = ml_dtypes.bfloat16

# K-block split per layer: (rows, bf16?) — order must match kernel rhs order.
_SPLITS = {
    "sa1": [[(6, 1)], [(32, 1)], [(32, 1)]],
    "sa2": [[(67, 0)], [(64, 1)], [(64, 1)]],
    "sa3": [[(128, 0), (3, 1)], [(128, 1)], [(128, 1)]],
    "sa4": [[(3, 1), (128, 1), (128, 1)], [(128, 1), (128, 1)],
            [(128, 1), (128, 1)]],
    "fp4": [[(128, 1)] * 6, [(128, 1), (128, 1)]],
    "fp3": [[(128, 1), (128, 1), (128, 0)], [(128, 1), (128, 1)]],
    "fp2": [[(128, 1), (128, 1), (64, 0)], [(128, 1), (128, 1)]],
    "fp1": [[(128, 1), (6, 1)], [(128, 1)]],
    "head1": [[(128, 1)]],
    "head2": [[(128, 1)]],
    "head3": [[(128, 1)]],
}
_COUT = {"sa1": [32, 32, 64], "sa2": [64, 64, 128], "sa3": [128, 128, 256],
         "sa4": [256, 256, 512], "fp4": [256, 256], "fp3": [256, 256],
         "fp2": [256, 128], "fp1": [128, 128], "head1": [128],
         "head2": [128], "head3": [CH * T]}

# ----------------------------------------------------------------- host math


def _fps(xyz, npoint):
    Bb, Nn, _ = xyz.shape
    dist = np.full((Bb, Nn), 1e10, np.float32)
    idxs = np.zeros((Bb, npoint), np.int64)
    last = np.zeros((Bb,), np.int64)
    ar = np.arange(Bb)
    for s in range(1, npoint):
        p = xyz[ar, last]
        d = ((xyz - p[:, None, :]) ** 2).sum(-1)
        dist = np.minimum(dist, d)
        last = dist.argmax(-1)
        idxs[:, s] = last
    return idxs


def _ball(new_xyz, xyz, r, k):
    d2 = ((new_xyz[:, None, :] - xyz[None, :, :]) ** 2).sum(-1)
    Nn = xyz.shape[0]
    keyv = np.where(d2 < r * r, np.arange(Nn)[None, :], Nn)
    part = np.partition(keyv, k - 1, axis=-1)[:, :k] if k < Nn else keyv
    order = np.sort(part, -1)[:, :k]
    first = order[:, :1]
    idx = np.where(order < Nn, order, np.where(first < Nn, first, 0))
    return idx.astype(np.int64)


def _interp_mat(unknown, known, n, m):
    d2 = ((unknown[:, None, :] - known[None, :, :]) ** 2).sum(-1)
    ii = np.argsort(d2, axis=-1, kind="stable")[:, :3]
    dsel = np.take_along_axis(d2, ii, -1).astype(np.float32)
    dist = np.sqrt(np.maximum(dsel, 0.0))
    w = 1.0 / (dist + 1e-8)
    w = (w / w.sum(-1, keepdims=True)).astype(np.float32)
    W = np.zeros((m, n), np.float32)
    ar = np.arange(n)
    for k in range(3):
        W[ii[:, k], ar] += w[:, k]
    return W


def _wrap16(idx, parts):
    n = idx.shape[0]
    w = idx.reshape(n // 16, 16).T.astype(np.int16)
    return np.tile(w, (parts // 16, 1))


def _prep_host(xyz, points, params):
    fidx1 = _fps(xyz, S1)
    l1x = np.take_along_axis(xyz, fidx1[..., None], axis=1)
    fidx2 = _fps(l1x, S2)
    l2x = np.take_along_axis(l1x, fidx2[..., None], axis=1)
    fidx3 = _fps(l2x, S3)
    l3x = np.take_along_axis(l2x, fidx3[..., None], axis=1)

    ptsT = np.transpose(points, (0, 2, 1))

    g1 = np.empty((B, 6, S1 * K1), np.float32)
    g2c = np.empty((B, 3, G2), np.float32)
    g3c = np.empty((B, 3, G3), np.float32)
    idx2w = np.empty((B, 4, 64, G2 // 64), np.int16)   # four quarters
    idx3w = np.empty((B, 2, 128, G3 // 32), np.int16)
    wi3 = np.empty((B, S3, S2), np.float32)
    wi2 = np.empty((B, S2, S1), np.float32)
    wi1 = np.empty((B, S1, N), np.float32)
    for b in range(B):
        i1 = _ball(l1x[b], xyz[b], R1, K1)
        gx = xyz[b][i1] - l1x[b][:, None, :]
        gn = ptsT[b][i1]
        g1[b] = np.concatenate([gx, gn], -1).transpose(2, 0, 1).reshape(6, -1)

        i2 = _ball(l2x[b], l1x[b], R2, K2)
        i2f = i2.reshape(-1)
        for qq in range(4):
            idx2w[b, qq] = _wrap16(i2f[qq * G2 // 4:(qq + 1) * G2 // 4], 64)
        g2c[b] = (l1x[b][i2] - l2x[b][:, None, :]).transpose(2, 0, 1).reshape(3, -1)

        i3 = _ball(l3x[b], l2x[b], R3, K3)
        g3c[b] = (l2x[b][i3] - l3x[b][:, None, :]).transpose(2, 0, 1).reshape(3, -1)
        i3f = i3.reshape(-1)
        idx3w[b, 0] = _wrap16(i3f[:G3 // 2], 128)
        idx3w[b, 1] = _wrap16(i3f[G3 // 2:], 128)

        wi3[b] = _interp_mat(l2x[b], l3x[b], S2, S3)
        wi2[b] = _interp_mat(l1x[b], l2x[b], S1, S2)
        wi1[b] = _interp_mat(xyz[b], l1x[b], N, S1)

    uf = np.concatenate([np.transpose(xyz, (0, 2, 1)), points], 1)

    wblobs = {}
    for nm, layers in _SPLITS.items():
        for li, blocks in enumerate(layers):
            W, g, bb = params[nm][li]
            W = np.asarray(W, np.float32) * np.asarray(g, np.float32)[:, None]
            WT = W.T.copy()
            if nm in ("sa2", "sa3") and li == 0:
                WT = np.concatenate([WT[3:], WT[:3]], 0)  # [feat; coord]
            r0 = 0
            for ki, (kb, isbf) in enumerate(blocks):
                blk = WT[r0:r0 + kb]
                wblobs[f"W_{nm}_{li}_{ki}"] = (
                    blk.astype(BF16) if isbf else blk.astype(np.float32))
                r0 += kb
            assert r0 == WT.shape[0], (nm, li, r0, WT.shape)
            bb = np.asarray(bb, np.float32)
            Cout = bb.shape[0]
            if Cout % 128 == 0 and Cout > 128:
                bmat = bb.reshape(Cout // 128, 128).T.copy()
            elif Cout < 128:
                bmat = np.tile(bb, 128 // Cout)[:128, None].copy()
            else:
                bmat = bb[:, None].copy()
            wblobs[f"b_{nm}_{li}"] = bmat

    per_core = []
    for c in range(NCORES):
        sl = slice(c * BPC, (c + 1) * BPC)
        m = {
            "g1": g1[sl].astype(BF16),
            "g2c": g2c[sl].astype(np.float32),
            "g3c": g3c[sl].astype(BF16),
            "idx2": idx2w[sl],
            "idx3": idx3w[sl],
            "l3x": l3x[sl].transpose(0, 2, 1).astype(BF16).copy(),
            "wi3": wi3[sl].astype(BF16),
            "wi2": wi2[sl].astype(BF16),
            "wi1": wi1[sl].reshape(BPC, 2, 128, N).astype(BF16),
            "uf": uf[sl].astype(BF16),
        }
        m.update(wblobs)
        per_core.append(m)
    return per_core


# ------------------------------------------------------------- device kernel

_BUILT = None
CNK = 512
CNB = 1024


def _build():
    import contextlib
    import concourse.mybir as mybir
    import concourse.tile as tile
    from concourse import bacc
    from concourse.masks import make_identity

    fp32 = mybir.dt.float32
    bf16 = mybir.dt.bfloat16
    i16 = mybir.dt.int16
    AX = mybir.AxisListType.X
    ALU = mybir.AluOpType
    ACTF = mybir.ActivationFunctionType

    nc = bacc.Bacc("TRN2", target_bir_lowering=False, debug=False)

    def din(name, shape, dt):
        return nc.dram_tensor(name, shape, dt, kind="ExternalInput").ap()

    g1_d = din("g1", [BPC, 6, S1 * K1], bf16)
    g2c_d = din("g2c", [BPC, 3, G2], fp32)
    g3c_d = din("g3c", [BPC, 3, G3], bf16)
    idx2_d = din("idx2", [BPC, 4, 64, G2 // 64], i16)
    idx3_d = din("idx3", [BPC, 2, 128, G3 // 32], i16)
    l3x_d = din("l3x", [BPC, 3, S3], bf16)
    wi3_d = din("wi3", [BPC, S3, S2], bf16)
    wi2_d = din("wi2", [BPC, S2, S1], bf16)
    wi1_d = din("wi1", [BPC, 2, 128, N], bf16)
    uf_d = din("uf", [BPC, 6, N], bf16)
    out_d = nc.dram_tensor("out", [BPC, CH * T, N], fp32,
                           kind="ExternalOutput").ap()
    import os
    dbg = os.environ.get("BASSDBG", "0") == "1"
    dbg_d = {}
    if dbg:
        for nm_, shp, dt_ in [("d_l1f", [64, S1], fp32),
                              ("d_l2f", [128, S2], fp32),
                              ("d_l3f", [128, 2, S3], bf16),
                              ("d_l4f", [128, 4, 1], fp32),
                              ("d_l3fn", [128, 2, S3], bf16),
                              ("d_l2fn", [128, 2, S2], bf16),
                              ("d_l1fn", [128, S1], bf16),
                              ("d_h1fp1", [128, N], bf16),
                              ("d_feat", [128, N], bf16),
                              ("d_g2f", [64, G2 // 2], fp32),
                              ("d_t1", [128, N], bf16)]:
            dbg_d[nm_] = nc.dram_tensor(nm_, shp, dt_,
                                        kind="ExternalOutput").ap()

    wdecl = {}
    for nm, layers in _SPLITS.items():
        wdecl[nm] = []
        for li, blocks in enumerate(layers):
            Cout = _COUT[nm][li]
            blks = [din(f"W_{nm}_{li}_{ki}", [kb, Cout],
                        bf16 if isbf else fp32)
                    for ki, (kb, isbf) in enumerate(blocks)]
            bshape = ([128, Cout // 128] if (Cout % 128 == 0 and Cout > 128)
                      else [min(Cout, 128), 1])
            wdecl[nm].append((blks, din(f"b_{nm}_{li}", bshape, fp32), Cout))

    with tile.TileContext(nc) as tc:
        with contextlib.ExitStack() as ctx:
            wpool = ctx.enter_context(tc.tile_pool(name="w", bufs=1))
            cpool = ctx.enter_context(tc.tile_pool(name="const", bufs=1))
            gin = ctx.enter_context(tc.tile_pool(name="gin", bufs=1))
            act16 = ctx.enter_context(tc.tile_pool(name="act16", bufs=2))
            g2fp = ctx.enter_context(tc.tile_pool(name="g2f", bufs=1))
            small = ctx.enter_context(tc.tile_pool(name="small", bufs=2))
            wip = ctx.enter_context(tc.tile_pool(name="wi", bufs=2))
            wi1p = ctx.enter_context(tc.tile_pool(name="wi1", bufs=1))
            outp = ctx.enter_context(tc.tile_pool(name="outp", bufs=1))
            pbig = ctx.enter_context(tc.tile_pool(name="pbig", bufs=3,
                                                  space="PSUM"))
            psml = ctx.enter_context(tc.tile_pool(name="psml", bufs=2,
                                                  space="PSUM"))

            identb = cpool.tile([128, 128], bf16, tag="idbf")
            make_identity(nc, identb[:])
            zbf = cpool.tile([128, CNB], bf16, tag="zbf")
            nc.vector.memset(zbf[:], 0.0)

            wt = {}
            for nm, layers in wdecl.items():
                wt[nm] = []
                for li, (blks, bd, Cout) in enumerate(layers):
                    tl = []
                    for ki, wd_ in enumerate(blks):
                        t = wpool.tile(wd_.shape, wd_.dtype,
                                       tag=f"W{nm}{li}{ki}")
                        nc.sync.dma_start(t[:], wd_)
                        tl.append(t)
                    bt = wpool.tile(bd.shape, fp32, tag=f"b{nm}{li}")
                    nc.sync.dma_start(bt[:], bd)
                    wt[nm].append((tl, bt, Cout))

            # two rotating activation buffers (16KB slots)
            _rot = [0]
            _galt = [0]

            def abuf(shape, dt):
                _rot[0] ^= 1
                tg = "actA" if _rot[0] else "actB"
                return act16.tile(shape, dt, tag=tg, name=tg)


            def dump(nm_, ap, c_):
                if dbg and c_ == 0:
                    nc.sync.dma_start(dbg_d[nm_], ap)
            def act_store(ps_ap, out_ap, bias_ap, alt, func, alpha):
                if func == ACTF.Lrelu:
                    nc.scalar.activation(out_ap, ps_ap, ACTF.Prelu,
                                         bias=bias_ap, alpha=float(alpha))
                elif func == ACTF.Relu and alt % 2 == 1 and out_ap.dtype == bf16:
                    nc.vector.scalar_tensor_tensor(
                        out_ap, ps_ap, bias_ap,
                        zbf[:ps_ap.shape[0], :ps_ap.shape[-1]],
                        op0=ALU.add, op1=ALU.max)
                elif func is None:
                    nc.scalar.activation(out_ap, ps_ap, ACTF.Copy)
                else:
                    nc.scalar.activation(out_ap, ps_ap, func,
                                         bias=bias_ap, alpha=alpha)

            def layer(nm, li, rhs_fns, n, out_fn, func="relu"):
                blks, bt, Cout = wt[nm][li]
                fn = {"relu": ACTF.Relu, "lrelu": ACTF.Lrelu,
                      "none": None}[func]
                alpha = 0.2 if func == "lrelu" else 0.0
                for mq in range((Cout + 127) // 128):
                    mw = min(128, Cout - mq * 128)
                    for cb in range((n + CNB - 1) // CNB):
                        c0 = cb * CNB
                        cw = min(CNB, n - c0)
                        ps = pbig.tile([128, CNB], fp32, tag="ps")
                        for sub in range(0, cw, CNK):
                            sw = min(CNK, cw - sub)
                            sl = slice(c0 + sub, c0 + sub + sw)
                            for ki, rf in enumerate(rhs_fns):
                                nc.tensor.matmul(
                                    ps[:mw, sub:sub + sw],
                                    blks[ki][:, mq * 128:mq * 128 + mw],
                                    rf(sl), start=(ki == 0),
                                    stop=(ki == len(rhs_fns) - 1))
                        bap = (bt[:mw, mq:mq + 1] if bt.shape[1] > 1
                               else bt[:mw, :])
                        act_store(ps[:mw, :cw], out_fn(mq, slice(c0, c0 + cw)),
                                  bap, _galt[0], fn, alpha)
                        _galt[0] += 1

            for c in range(BPC):
                # ======== SA1 ========
                g1t = gin.tile([6, S1 * K1], bf16, tag="gbuf")
                nc.sync.dma_start(g1t[:], g1_d[c])
                h1 = abuf([32, N], bf16)
                layer("sa1", 0, [lambda sl: g1t[:, sl]], N,
                      lambda mq, sl: h1[:, sl])
                h2 = abuf([32, N], bf16)
                layer("sa1", 1, [lambda sl: h1[:, sl]], N,
                      lambda mq, sl: h2[:, sl])
                h3 = abuf([64, N], bf16)
                layer("sa1", 2, [lambda sl: h2[:, sl]], N,
                      lambda mq, sl: h3[:, sl])
                l1fb = small.tile([64, S1], bf16, tag="l1fb")
                nc.vector.tensor_reduce(
                    l1fb[:], h3[:].rearrange("p (s k) -> p s k", k=K1),
                    axis=AX, op=ALU.max)
                l1f = small.tile([64, S1], fp32, tag="l1f")
                nc.vector.tensor_copy(l1f[:], l1fb[:])
                dump("d_l1f", l1f[:], c)

                # ======== SA2 ========
                i2t = small.tile([64, 4, G2 // 64], i16, tag="i2")
                nc.sync.dma_start(i2t[:],
                                  idx2_d[c].rearrange("h p n -> p h n"))
                h1 = abuf([64, G2], bf16)
                blks2, bt2, _ = wt["sa2"][0]
                for hf in range(4):
                    off = hf * (G2 // 4)
                    g2f = g2fp.tile([67, G2 // 4], fp32, tag="g2f",
                                    bufs=2)
                    nc.gpsimd.ap_gather(
                        g2f[0:64, :].rearrange("p (n d) -> p n d", d=1),
                        l1f[:].rearrange("p (n d) -> p n d", d=1),
                        i2t[:, hf, :], channels=64, num_elems=S1, d=1,
                        num_idxs=G2 // 4)
                    nc.sync.dma_start(g2f[64:67, :],
                                      g2c_d[c, :, off:off + G2 // 4])
                    if hf == 0:
                        dump("d_g2f", g2f[0:64, :], c)
                    for cb in range(2):
                        ps = pbig.tile([128, CNB], fp32, tag="ps")
                        for sub in range(2):
                            s0 = cb * CNB + sub * CNK
                            nc.tensor.matmul(
                                ps[:64, sub * CNK:(sub + 1) * CNK],
                                blks2[0][:], g2f[:, s0:s0 + CNK],
                                start=True, stop=True)
                        act_store(ps[:64, :], h1[:, off + cb * CNB:
                                                 off + (cb + 1) * CNB],
                                  bt2[:64, :], _galt[0], ACTF.Relu, 0.0)
                        _galt[0] += 1
                h2 = abuf([64, G2], bf16)
                layer("sa2", 1, [lambda sl: h1[:, sl]], G2,
                      lambda mq, sl: h2[:, sl])
                h3 = abuf([128, G2], bf16)
                layer("sa2", 2, [lambda sl: h2[:, sl]], G2,
                      lambda mq, sl: h3[:, sl])
                l2fb = small.tile([128, S2], bf16, tag="l2fb")
                nc.vector.tensor_reduce(
                    l2fb[:], h3[:].rearrange("p (s k) -> p s k", k=K2),
                    axis=AX, op=ALU.max)
                l2f = small.tile([128, S2], fp32, tag="l2f")
                nc.vector.tensor_copy(l2f[:], l2fb[:])
                dump("d_l2f", l2f[:], c)

                # ======== SA3 ========
                i3t = small.tile([128, 2, G3 // 32], i16, tag="i3")
                nc.sync.dma_start(i3t[:],
                                  idx3_d[c].rearrange("h p n -> p h n"))
                g3ct = small.tile([3, G3], bf16, tag="g3c")
                nc.sync.dma_start(g3ct[:], g3c_d[c])
                h1 = abuf([128, G3], bf16)
                blks3, bt3, _ = wt["sa3"][0]
                for hf in range(2):
                    off = hf * (G3 // 2)
                    g3f = g2fp.tile([128, G3 // 2], fp32, tag="g3f",
                                    name="g3f")
                    nc.gpsimd.ap_gather(
                        g3f[:].rearrange("p (n d) -> p n d", d=1),
                        l2f[:].rearrange("p (n d) -> p n d", d=1),
                        i3t[:, hf, :], channels=128, num_elems=S2, d=1,
                        num_idxs=G3 // 2)
                    for cb in range(G3 // 2 // CNB):
                        ps = pbig.tile([128, CNB], fp32, tag="ps")
                        for sub in range(2):
                            s0 = cb * CNB + sub * CNK
                            pslc = ps[:, sub * CNK:(sub + 1) * CNK]
                            nc.tensor.matmul(pslc, blks3[0][:],
                                             g3f[:, s0:s0 + CNK],
                                             start=True, stop=False)
                            nc.tensor.matmul(pslc, blks3[1][:],
                                             g3ct[:, off + s0:off + s0 + CNK],
                                             start=False, stop=True)
                        act_store(ps[:, :],
                                  h1[:, off + cb * CNB:off + (cb + 1) * CNB],
                                  bt3[:, :], _galt[0], ACTF.Relu, 0.0)
                        _galt[0] += 1
                h2 = abuf([128, G3], bf16)
                layer("sa3", 1, [lambda sl: h1[:, sl]], G3,
                      lambda mq, sl: h2[:, sl])
                h3 = abuf([128, 2, G3], bf16)
                layer("sa3", 2, [lambda sl: h2[:, sl]], G3,
                      lambda mq, sl: h3[:, mq, sl])
                l3f = small.tile([128, 2, S3], bf16, tag="l3f")
                for q in range(2):
                    nc.vector.tensor_reduce(
                        l3f[:, q, :],
                        h3[:, q, :].rearrange("p (s k) -> p s k", k=K3),
                        axis=AX, op=ALU.max)
                dump("d_l3f", l3f[:], c)

                # ======== SA4 ========
                l3xt = small.tile([3, S3], bf16, tag="l3x")
                nc.sync.dma_start(l3xt[:], l3x_d[c])
                h1 = small.tile([128, 2, S3], bf16, tag="s4h1")
                layer("sa4", 0, [lambda sl: l3xt[:, sl],
                                 lambda sl: l3f[:, 0, sl],
                                 lambda sl: l3f[:, 1, sl]], S3,
                      lambda mq, sl: h1[:, mq, sl])
                h2 = small.tile([128, 2, S3], bf16, tag="s4h2")
                layer("sa4", 1, [lambda sl: h1[:, 0, sl],
                                 lambda sl: h1[:, 1, sl]], S3,
                      lambda mq, sl: h2[:, mq, sl])
                h4 = small.tile([128, 4, S3], bf16, tag="s4h3")
                layer("sa4", 2, [lambda sl: h2[:, 0, sl],
                                 lambda sl: h2[:, 1, sl]], S3,
                      lambda mq, sl: h4[:, mq, sl])
                l4f = small.tile([128, 4, 1], fp32, tag="l4f")
                for q in range(4):
                    nc.vector.tensor_reduce(
                        l4f[:, q, :], h4[:, q, :], axis=AX, op=ALU.max)
                dump("d_l4f", l4f[:], c)

                # ======== FP4 ========
                b4 = small.tile([128, 4, S3], bf16, tag="b4")
                for q in range(4):
                    nc.vector.tensor_scalar_add(
                        b4[:, q, :], zbf[:, :S3], l4f[:, q, :])
                h1 = small.tile([128, 2, S3], bf16, tag="f4h1")
                layer("fp4", 0,
                      [lambda sl, q=q: b4[:, q, sl] for q in range(4)]
                      + [lambda sl: l3f[:, 0, sl], lambda sl: l3f[:, 1, sl]],
                      S3, lambda mq, sl: h1[:, mq, sl])
                l3fn = small.tile([128, 2, S3], bf16, tag="l3fn")
                layer("fp4", 1, [lambda sl: h1[:, 0, sl],
                                 lambda sl: h1[:, 1, sl]], S3,
                      lambda mq, sl: l3fn[:, mq, sl])
                dump("d_l3fn", l3fn[:], c)

                # ======== FP3 ========
                wi3t = wip.tile([S3, S2], bf16, tag="wi3")
                nc.sync.dma_start(wi3t[:], wi3_d[c])
                l3fT = small.tile([S3, 256], bf16, tag="l3fT")
                for q in range(2):
                    pt = psml.tile([128, CNK], bf16, tag="psm")
                    nc.tensor.transpose(pt[:S3, :128], l3fn[:, q, :],
                                        identb[:])
                    nc.scalar.activation(l3fT[:, q * 128:(q + 1) * 128],
                                         pt[:S3, :128], ACTF.Copy)
                it3 = small.tile([128, 2, S2], bf16, tag="it3")
                for q in range(2):
                    ps = psml.tile([128, CNK], fp32, tag="psm")
                    nc.tensor.matmul(ps[:, :S2],
                                     l3fT[:, q * 128:(q + 1) * 128],
                                     wi3t[:], start=True, stop=True)
                    nc.scalar.activation(it3[:, q, :], ps[:, :S2], ACTF.Copy)
                h1 = small.tile([128, 2, S2], bf16, tag="f3h1")
                layer("fp3", 0, [lambda sl: it3[:, 0, sl],
                                 lambda sl: it3[:, 1, sl],
                                 lambda sl: l2f[:, sl]], S2,
                      lambda mq, sl: h1[:, mq, sl])
                l2fn = small.tile([128, 2, S2], bf16, tag="l2fn")
                layer("fp3", 1, [lambda sl: h1[:, 0, sl],
                                 lambda sl: h1[:, 1, sl]], S2,
                      lambda mq, sl: l2fn[:, mq, sl])
                dump("d_l2fn", l2fn[:], c)

                # ======== FP2 ========
                wi2t = wip.tile([S2, S1], bf16, tag="wi2")
                nc.sync.dma_start(wi2t[:], wi2_d[c])
                l2fT = small.tile([S2, 256], bf16, tag="l2fT")
                for q in range(2):
                    pt = psml.tile([128, CNK], bf16, tag="psm")
                    nc.tensor.transpose(pt[:S2, :128], l2fn[:, q, :],
                                        identb[:])
                    nc.scalar.activation(l2fT[:, q * 128:(q + 1) * 128],
                                         pt[:S2, :128], ACTF.Copy)
                it2 = small.tile([128, 2, S1], bf16, tag="it2")
                for q in range(2):
                    ps = psml.tile([128, CNK], fp32, tag="psm")
                    nc.tensor.matmul(ps[:, :S1],
                                     l2fT[:, q * 128:(q + 1) * 128],
                                     wi2t[:], start=True, stop=True)
                    nc.scalar.activation(it2[:, q, :], ps[:, :S1], ACTF.Copy)
                h1 = small.tile([128, 2, S1], bf16, tag="f2h1")
                layer("fp2", 0, [lambda sl: it2[:, 0, sl],
                                 lambda sl: it2[:, 1, sl],
                                 lambda sl: l1f[:, sl]], S1,
                      lambda mq, sl: h1[:, mq, sl])
                l1fn = small.tile([128, S1], bf16, tag="l1fn")
                layer("fp2", 1, [lambda sl: h1[:, 0, sl],
                                 lambda sl: h1[:, 1, sl]], S1,
                      lambda mq, sl: l1fn[:, sl])
                dump("d_l1fn", l1fn[:], c)

                # ======== FP1 (interp matmul folded into layer 0) ========
                blks, b0t, _ = wt["fp1"][0]
                w0a, w0b = blks
                psy = psml.tile([128, CNK], fp32, tag="psm")
                nc.tensor.matmul(psy[:, :S1], w0a[:], l1fn[:],
                                 start=True, stop=True)
                ysb = small.tile([128, S1], bf16, tag="ysb")
                nc.scalar.activation(ysb[:], psy[:, :S1], ACTF.Copy)
                yT = small.tile([128, 2, 128], bf16, tag="yT")
                for q in range(2):
                    pt = psml.tile([128, CNK], bf16, tag="psm")
                    nc.tensor.transpose(pt[:, :128],
                                        ysb[:, q * 128:(q + 1) * 128],
                                        identb[:])
                    nc.scalar.activation(yT[:, q, :], pt[:, :128], ACTF.Copy)
                wi1t = wi1p.tile([128, 2, N // 2], bf16, tag="wi1")
                nc.sync.dma_start(wi1t[:], wi1_d[c, :, :, :N // 2])
                wi1u = wi1p.tile([128, 2, N // 2], bf16, tag="wi1b")
                nc.sync.dma_start(wi1u[:], wi1_d[c, :, :, N // 2:])
                uft = gin.tile([6, N], bf16, tag="gbuf")
                nc.sync.dma_start(uft[:], uf_d[c])
                h1 = abuf([128, N], bf16)
                for cb in range(N // CNK):
                    sl = slice(cb * CNK, (cb + 1) * CNK)
                    wsrc = wi1t if cb < 8 else wi1u
                    slw = slice((cb % 8) * CNK, (cb % 8 + 1) * CNK)
                    ps = pbig.tile([128, CNK], fp32, tag="ps")
                    nc.tensor.matmul(ps[:, :], yT[:, 0, :], wsrc[:, 0, slw],
                                     start=True, stop=False)
                    nc.tensor.matmul(ps[:, :], yT[:, 1, :], wsrc[:, 1, slw],
                                     start=False, stop=False)
                    nc.tensor.matmul(ps[:, :], w0b[:], uft[:, sl],
                                     start=False, stop=True)
                    act_store(ps[:, :], h1[:, sl], b0t[:, :], cb,
                              ACTF.Relu, 0.0)
                dump("d_h1fp1", h1[:], c)
                feat = abuf([128, N], bf16)
                layer("fp1", 1, [lambda sl: h1[:, sl]], N,
                      lambda mq, sl: feat[:, sl])
                dump("d_feat", feat[:], c)

                # ======== heads ========
                t1 = abuf([128, N], bf16)
                layer("head1", 0, [lambda sl: feat[:, sl]], N,
                      lambda mq, sl: t1[:, sl], func="lrelu")
                dump("d_t1", t1[:], c)
                t2 = abuf([128, N], bf16)
                layer("head2", 0, [lambda sl: t1[:, sl]], N,
                      lambda mq, sl: t2[:, sl], func="lrelu")
                w3 = wt["head3"][0][0][0]
                for cb in range(N // CNB):
                    bsl = slice(cb * CNB, (cb + 1) * CNB)
                    ps = pbig.tile([128, CNB], fp32, tag="ps")
                    for sub in range(2):
                        s0 = cb * CNB + sub * CNK
                        nc.tensor.matmul(ps[:, sub * CNK:(sub + 1) * CNK],
                                         w3[:, 0:128], t2[:, s0:s0 + CNK],
                                         start=True, stop=True)
                    osb = outp.tile([128, CNB], fp32, tag="osb")
                    nc.scalar.activation(osb[:], ps[:, :], ACTF.Copy)
                    nc.sync.dma_start(out_d[c, 0:128, bsl], osb[:])
                    for sub in range(2):
                        s0 = cb * CNB + sub * CNK
                        ps2 = psml.tile([128, CNK], fp32, tag="psm",
                                        name="ps2")
                        nc.tensor.matmul(ps2[:7, :], w3[:, 128:135],
                                         t2[:, s0:s0 + CNK],
                                         start=True, stop=True)
                        osb2 = outp.tile([7, CNK], fp32, tag="osb2",
                                         name="osb2")
                        nc.vector.tensor_copy(osb2[:], ps2[:7, :])
                        nc.sync.dma_start(out_d[c, 128:135, s0:s0 + CNK],
                                          osb2[:])

    nc.compile()
    return nc


def _get_built():
    global _BUILT
    if _BUILT is None:
        _BUILT = _build()
    return _BUILT


def run_device(per_core, trace=False, tmpdir=None):
    from concourse.bass_utils import run_bass_kernel_spmd

    nc = _get_built()
    res = run_bass_kernel_spmd(nc, per_core, core_ids=list(range(NCORES)),
                               trace=trace, tmpdir=tmpdir)
    out = np.concatenate([r["out"] for r in res.results], 0)
    return out, res


def kernel(xyz, points, params):
    xyz = np.asarray(xyz, np.float32)
    points = np.asarray(points, np.float32)
    per_core = _prep_host(xyz, points, params)
    out, _ = run_device(per_core)
    bat = out.transpose(0, 2, 1).reshape(B, N, CH, T)
    return bat[:, :, :2, :], bat[:, :, 2:-20, :], bat[:, :, -20:, :]
